# revision 6
# baseline (speedup 1.0000x reference)
"""BiGAT (2-layer GAT, PyG-style with self-loops) on 8 Trainium2 NeuronCores.

Strategy: partition nodes (and their incoming edges) by destination across 8
cores. Nodes are permuted so that every 125-node dst block carries a near-equal
edge count (greedy degree balancing), which makes the per-block chunk count
uniform and small. Edges are sorted by dst and padded to a uniform
blocks-x-chunks structure so a single SPMD program serves all cores.

Per layer:
  node stage : xh = x @ W (PE), attention dot-products via block-diagonal
               matmul; packed per-node rows [xh | a_src | a_dst | pad] written
               to a local DRAM table; AllGather replicates the table.
  edge stage : dma_gather of full rows by src (gives xh+a_src) and of the
               [a_src|a_dst] suffix by dst-local from the local table;
               e = lrelu(a_src+a_dst); ex = exp(e)  (softmax max-shift is
               skipped -- scores are O(10) so exp cannot overflow, and softmax
               is shift-invariant); msg = xh_src * ex; one-hot matmul
               scatter-adds [msg | ex] into PSUM per 125-node block; epilogue
               divides by the summed ex (denominator), adds bias.

Gathers are batched: G=5 blocks form a super-block whose indices are laid out
contiguously per gather table, so each super-block needs only 4 dma_gather
calls (src-half-A, src-half-B, dst x2) -- amortizing the ~1us SWDGE
fixed overhead per call.

dma_gather constraints honored: int16 indices (src tables split into two
<=25000-row halves; dst uses core-local indices), row strides and elem sizes
multiples of 256B, indices wrapped [16, n/16] and replicated to 128 partitions.
"""
import sys

sys.path.insert(0, "/opt/trn_rl_repo")

import heapq
import numpy as np

from concourse import bass, mybir
import concourse.bacc as bacc
import concourse.tile as tile
from concourse.masks import make_identity

F32 = mybir.dt.float32
I16 = mybir.dt.int16
BF16 = True                      # table/gather dtype (False -> float32)
TD = mybir.dt.bfloat16 if BF16 else F32
import ml_dtypes
TNP = ml_dtypes.bfloat16 if BF16 else np.float32

# ---------------- problem constants (hardcoded per contract) ----------------
N_NODES = 50000
N_EDGES = 800000
IN_C, HID_C, OUT_C, HEADS = 128, 16, 64, 8
NEG_SLOPE = 0.2
N_CORES = 8

# ---------------- sharding / tiling parameters ----------------
BLK = 125       # dst nodes per edge-stage block (<=128 for one-hot)
P1W = 256 if BF16 else 192   # [xh(128) | a_src(8) | a_dst(8) | pad]
P2W = 128                    # [xh2(64) | a_src2(1) | a_dst2(1) | pad]
G = 5           # blocks per super-block (per merged gather call group)
MAXI = 8192     # max idxs per dma_gather call
EPS = 1e-16


def _wrap16(idx):
    """[L] int array -> dma_gather wrapped layout [128, L//16] int16."""
    L = len(idx)
    w = idx.reshape(L // 16, 16).T
    return np.tile(w, (8, 1)).astype(np.int16)


def _balance_blocks(dst, n_nodes, nblk_tot):
    """Greedy assign nodes to blocks of exactly BLK nodes, equalizing the
    per-block incoming-edge count. Returns perm (new position -> old node)."""
    deg = np.bincount(dst, minlength=n_nodes)
    order = np.argsort(-deg, kind="stable")
    fill = np.zeros(nblk_tot, np.int32)
    perm = np.empty(n_nodes, np.int64)
    heap = [(0, b) for b in range(nblk_tot)]
    heapq.heapify(heap)
    for node in order:
        load, b = heapq.heappop(heap)
        perm[b * BLK + fill[b]] = node
        fill[b] += 1
        load += int(deg[node])
        if fill[b] < BLK:
            heapq.heappush(heap, (load, b))
    assert (fill == BLK).all()
    return perm


def _host_prep(x, edge_index, W1, att_src1, att_dst1, b1, W2, att_src2, att_dst2, b2,
               n_nodes=N_NODES, n_cores=N_CORES):
    """Sort/pad edges, build per-core input maps and compile-time params."""
    NP = n_nodes // n_cores
    NB = NP // BLK
    assert NB * BLK == NP and NB % G == 0
    NSB = NB // G
    HALF = n_nodes // 2
    assert HALF < 32768 and NP < 32768

    src0 = np.concatenate([np.asarray(edge_index[0]), np.arange(n_nodes)]).astype(np.int64)
    dst0 = np.concatenate([np.asarray(edge_index[1]), np.arange(n_nodes)]).astype(np.int64)

    nblk_tot = n_cores * NB
    perm = _balance_blocks(dst0, n_nodes, nblk_tot)     # new pos -> old node
    old2new = np.empty(n_nodes, np.int64)
    old2new[perm] = np.arange(n_nodes)

    src = old2new[src0]
    dst = old2new[dst0]
    order = np.argsort(dst, kind="stable")
    src, dst = src[order], dst[order]

    blk_of = dst // BLK
    # within each dst-block, put src<HALF ("a") edges first
    order2 = np.lexsort((src >= HALF, blk_of))
    src, dst = src[order2], dst[order2]
    is_b = src >= HALF
    cnt_a = np.bincount(blk_of[order2], weights=~is_b, minlength=nblk_tot).astype(np.int64)
    cnt_b = np.bincount(blk_of[order2], weights=is_b, minlength=nblk_tot).astype(np.int64)
    starts = np.concatenate([[0], np.cumsum(cnt_a + cnt_b)]).astype(np.int64)
    Ka = int(np.ceil(cnt_a.max() / 128))
    Kb = int(np.ceil(cnt_b.max() / 128))
    K = Ka + Kb

    # per-block padded arrays in [a-pad | b-pad] chunk order
    srcA = np.zeros((nblk_tot, Ka * 128), np.int64)      # pad -> row 0
    srcB = np.zeros((nblk_tot, Kb * 128), np.int64)
    dstL = np.zeros((nblk_tot, K * 128), np.int64)       # dst local to core
    dloc = np.full((nblk_tot, K * 128), 999.0, np.float32)  # dst local to block
    for b in range(nblk_tot):
        na, nb_ = int(cnt_a[b]), int(cnt_b[b])
        s = starts[b]
        core = b // NB
        srcA[b, :na] = src[s:s + na]
        srcB[b, :nb_] = src[s + na:s + na + nb_] - HALF
        dstL[b, :na] = dst[s:s + na] - core * NP
        dstL[b, Ka * 128:Ka * 128 + nb_] = dst[s + na:s + na + nb_] - core * NP
        dloc[b, :na] = dst[s:s + na] - b * BLK
        dloc[b, Ka * 128:Ka * 128 + nb_] = dst[s + na:s + na + nb_] - b * BLK

    # shared (replicated) weights
    AA1 = np.zeros((128, 16), np.float32)
    asrc1 = np.asarray(att_src1, np.float32)
    adst1 = np.asarray(att_dst1, np.float32)
    for h in range(HEADS):
        AA1[16 * h:16 * (h + 1), h] = asrc1[h]
        AA1[16 * h:16 * (h + 1), 8 + h] = adst1[h]
    AA2 = np.stack([np.asarray(att_src2, np.float32)[0],
                    np.asarray(att_dst2, np.float32)[0]], axis=1)  # [64, 2]
    shared = {
        "W1": np.asarray(W1, np.float32),
        "AA1": AA1,
        "B1": np.tile(np.asarray(b1, np.float32), (128, 1)),
        "W2": np.asarray(W2, np.float32),
        "AA2": AA2,
        "B2": np.tile(np.asarray(b2, np.float32), (128, 1)),
        "IOTA": np.tile(np.arange(128), (128, 1)).astype(TNP),
    }

    xT = np.ascontiguousarray(np.asarray(x, np.float32).T)  # [128, N] (old order)

    in_maps = []
    for c in range(n_cores):
        lo = c * NB
        # super-block idx layout: [A(b0..bG-1) | B(b0..) | dst(b0..)] wrapped
        idx = np.stack([
            np.concatenate(
                [_wrap16(srcA[lo + s * G + g]) for g in range(G)] +
                [_wrap16(srcB[lo + s * G + g]) for g in range(G)] +
                [_wrap16(dstL[lo + s * G + g]) for g in range(G)], axis=1)
            for s in range(NSB)])
        dl = np.stack([
            np.concatenate(
                [dloc[lo + s * G + g].reshape(K, 128).T for g in range(G)], axis=1)
            for s in range(NSB)])                        # [NSB, 128, G*K]
        m = dict(shared)
        m["xT"] = np.ascontiguousarray(xT[:, perm[c * NP:(c + 1) * NP]])
        m["IDX"] = np.ascontiguousarray(idx)
        m["DLOC"] = np.ascontiguousarray(dl.astype(TNP))
        in_maps.append(m)

    prm = dict(NP=NP, NB=NB, NSB=NSB, K=K, Ka=Ka, Kb=Kb,
               n_nodes=n_nodes, n_cores=n_cores, HALF=HALF, perm=perm)
    return in_maps, prm


def _build_program(prm, repeat=1):
    NP, NSB, K, Ka, Kb = prm["NP"], prm["NSB"], prm["K"], prm["Ka"], prm["Kb"]
    HALF = prm["HALF"]
    n_nodes, n_cores = prm["n_nodes"], prm["n_cores"]
    RG = [list(range(n_cores))]
    CW = G * (Ka + Kb + K) * 8  # idx tensor cols per super-block

    nc = bacc.Bacc("TRN2", target_bir_lowering=False, debug=False,
                   num_devices=n_cores, num_swdge_queues=4)
    qn = [0]  # round-robin SWDGE queue assignment for gathers

    def next_q():
        qn[0] += 1
        return qn[0] % 4

    # inputs
    xT = nc.dram_tensor("xT", [128, NP], F32, kind="ExternalInput")
    W1 = nc.dram_tensor("W1", [128, 128], F32, kind="ExternalInput")
    AA1 = nc.dram_tensor("AA1", [128, 16], F32, kind="ExternalInput")
    B1 = nc.dram_tensor("B1", [128, 128], F32, kind="ExternalInput")
    W2 = nc.dram_tensor("W2", [128, 64], F32, kind="ExternalInput")
    AA2 = nc.dram_tensor("AA2", [64, 2], F32, kind="ExternalInput")
    B2 = nc.dram_tensor("B2", [128, 64], F32, kind="ExternalInput")
    IOTA = nc.dram_tensor("IOTA", [128, 128], TD, kind="ExternalInput")
    IDX = nc.dram_tensor("IDX", [NSB, 128, CW], I16, kind="ExternalInput")
    DLOC = nc.dram_tensor("DLOC", [NSB, 128, G * K], TD, kind="ExternalInput")
    OUT = nc.dram_tensor("out", [NP, OUT_C], F32, kind="ExternalOutput")
    # internal DRAM
    P1L = nc.dram_tensor("P1L", [NP, P1W], TD)
    P1F = nc.dram_tensor("P1F", [n_nodes, P1W], TD, addr_space="Shared")
    P2L = nc.dram_tensor("P2L", [NP, P2W], TD)
    P2F = nc.dram_tensor("P2F", [n_nodes, P2W], TD, addr_space="Shared")

    mm = mybir.AluOpType
    ACT = mybir.ActivationFunctionType

    def gcalls(ix, out3, table, col0, nchunk, elem, estep=None):
        """Issue merged dma_gather calls covering nchunk chunks (<=MAXI idxs
        per call), indices at ix[:, col0*8:...], output rows out3[:, 0:nchunk, :]."""
        step = MAXI // 128
        for c0 in range(0, nchunk, step):
            c1 = min(c0 + step, nchunk)
            nc.gpsimd.dma_gather(
                out_ap=out3[:, c0:c1, :], in_ap=table,
                idxs_ap=ix[:, (col0 + c0) * 8:(col0 + c1) * 8],
                num_idxs=(c1 - c0) * 128,
                num_idxs_reg=(c1 - c0) * 128, elem_size=elem,
                elem_step=estep, queue_num=next_q())

    from contextlib import ExitStack
    with tile.TileContext(nc) as tc, ExitStack() as ctx:
        cst = ctx.enter_context(tc.tile_pool(name="cst", bufs=1))
        W1t = cst.tile([128, 128], F32)
        AA1t = cst.tile([128, 16], F32)
        B1t = cst.tile([128, 128], F32)
        W2t = cst.tile([128, 64], F32)
        AA2t = cst.tile([64, 2], F32)
        B2t = cst.tile([128, 64], F32)
        IOTAt = cst.tile([128, 128], TD)
        IDENT = cst.tile([128, 128], F32)
        for t, d in ((W1t, W1), (AA1t, AA1), (B1t, B1), (W2t, W2),
                     (AA2t, AA2), (B2t, B2), (IOTAt, IOTA)):
            nc.sync.dma_start(out=t[:], in_=d[:, :])
        make_identity(nc, IDENT[:])

        # body may be repeated for differential benchmarking
        for _rep in range(repeat):
            # ---------------- stage A: L1 node stage ----------------
            with nc.named_scope("nodeA"), \
                 tc.tile_pool(name="pa", bufs=3) as pa, \
                 tc.tile_pool(name="ppa", bufs=3, space="PSUM") as ppa:
                for c0 in range(0, NP, 128):
                    nn = min(128, NP - c0)
                    xt = pa.tile([128, 128], F32, tag="xt")
                    nc.sync.dma_start(out=xt[:, :nn], in_=xT[:, c0:c0 + nn])
                    pm = ppa.tile([128, 128], F32, tag="pp")
                    nc.tensor.matmul(pm[:, :nn], lhsT=W1t[:], rhs=xt[:, :nn],
                                     start=True, stop=True)
                    xhT = pa.tile([128, 128], F32, tag="xhT")
                    nc.vector.tensor_copy(out=xhT[:, :nn], in_=pm[:, :nn])
                    pm2 = ppa.tile([16, 128], F32, tag="pp")
                    nc.tensor.matmul(pm2[:, :nn], lhsT=AA1t[:], rhs=xhT[:, :nn],
                                     start=True, stop=True)
                    aaT = pa.tile([16, 128], F32, tag="aaT")
                    nc.vector.tensor_copy(out=aaT[:, :nn], in_=pm2[:, :nn])
                    pt = ppa.tile([128, 128], F32, tag="pp")
                    nc.tensor.transpose(pt[:nn, :], xhT[:, :nn], IDENT[:])
                    xh = pa.tile([128, 128], TD, tag="xh")
                    nc.vector.tensor_copy(out=xh[:nn, :], in_=pt[:nn, :])
                    pt2 = ppa.tile([128, 16], F32, tag="pp")
                    nc.tensor.transpose(pt2[:nn, :], aaT[:, :nn], IDENT[:16, :16])
                    aa = pa.tile([128, P1W - 128], TD, tag="aa")
                    nc.vector.memset(aa[:, 16:], 0.0)
                    nc.vector.tensor_copy(out=aa[:nn, :16], in_=pt2[:nn, :])
                    nc.sync.dma_start(out=P1L[c0:c0 + nn, 0:128], in_=xh[:nn, :])
                    nc.sync.dma_start(out=P1L[c0:c0 + nn, 128:P1W], in_=aa[:nn, :])

            with nc.named_scope("ag1"):
                nc.gpsimd.collective_compute(
                    "AllGather", mm.bypass, replica_groups=RG,
                    ins=[P1L[:, :]], outs=[P1F[:, :]])

            # ---------------- L1 edge stage (+ fused L2 node stage) ----------------
            with nc.named_scope("edge1"), \
                 tc.tile_pool(name="gma", bufs=2) as gmap, \
                 tc.tile_pool(name="gmb", bufs=2) as gmbp, \
                 tc.tile_pool(name="gad", bufs=2) as gadp, \
                 tc.tile_pool(name="off", bufs=2) as offp, \
                 tc.tile_pool(name="sml", bufs=3) as sml, \
                 tc.tile_pool(name="sp", bufs=3) as spp, \
                 tc.tile_pool(name="hb", bufs=2) as hbp, \
                 tc.tile_pool(name="a2", bufs=2) as a2p, \
                 tc.tile_pool(name="ps1", bufs=2, space="PSUM") as ps1p, \
                 tc.tile_pool(name="psa2", bufs=2, space="PSUM") as psa2p:

                for sb in range(NSB):
                    ix = offp.tile([128, CW], I16, tag="ix")
                    nc.sync.dma_start(out=ix[:], in_=IDX[sb, :, :])
                    gmA = gmap.tile([128, G * Ka * P1W], TD, tag="gmA")
                    gmA3 = gmA[:].rearrange("p (r w) -> p r w", w=P1W)
                    gcalls(ix, gmA3, P1F[0:HALF, :], 0, G * Ka, P1W)
                    gmB = gmbp.tile([128, G * Kb * P1W], TD, tag="gmB")
                    gmB3 = gmB[:].rearrange("p (r w) -> p r w", w=P1W)
                    gcalls(ix, gmB3, P1F[HALF:n_nodes, :], G * Ka, G * Kb, P1W)
                    gad = gadp.tile([128, G * K * (P1W - 128)], TD, tag="gad")
                    gad3 = gad[:].rearrange("p (r w) -> p r w", w=P1W - 128)
                    gcalls(ix, gad3, P1L[:, 128:P1W], G * (Ka + Kb), G * K,
                           P1W - 128, estep=P1W)
                    dl = sml.tile([128, G * K], TD, tag="dl")
                    nc.sync.dma_start(out=dl[:], in_=DLOC[sb, :, :])

                    for bi in range(G):
                        ea = sml.tile([128, K * 8], F32, tag="ea")
                        ea3 = ea[:].rearrange("p (r w) -> p r w", w=8)
                        # e = a_src[src] + a_dst[dst]; a-chunks then b-chunks
                        nc.vector.tensor_tensor(
                            out=ea3[:, 0:Ka, :],
                            in0=gmA3[:, bi * Ka:(bi + 1) * Ka, 128:136],
                            in1=gad3[:, bi * K:bi * K + Ka, 8:16], op=mm.add)
                        nc.vector.tensor_tensor(
                            out=ea3[:, Ka:K, :],
                            in0=gmB3[:, bi * Kb:(bi + 1) * Kb, 128:136],
                            in1=gad3[:, bi * K + Ka:(bi + 1) * K, 8:16], op=mm.add)
                        tl = sml.tile([128, K * 8], F32, tag="tl")
                        nc.vector.tensor_scalar_mul(tl[:], ea[:], NEG_SLOPE)
                        nc.vector.tensor_tensor(out=ea[:], in0=ea[:], in1=tl[:],
                                                op=mm.max)
                        # ex -> overwrite the gathered a_src slots
                        nc.scalar.activation(
                            out=gmA3[:, bi * Ka:(bi + 1) * Ka, 128:136],
                            in_=ea3[:, 0:Ka, :], func=ACT.Exp)
                        nc.scalar.activation(
                            out=gmB3[:, bi * Kb:(bi + 1) * Kb, 128:136],
                            in_=ea3[:, Ka:K, :], func=ACT.Exp)
                        ps = ps1p.tile([128, 136], F32, tag="ps")
                        for k in range(K):
                            if k < Ka:
                                ck = gmA3[:, bi * Ka + k:bi * Ka + k + 1, :]
                            else:
                                kk = bi * Kb + (k - Ka)
                                ck = gmB3[:, kk:kk + 1, :]
                            S = spp.tile([128, 128], TD, tag="S")
                            nc.vector.tensor_tensor(
                                out=S[:], in0=IOTAt[:],
                                in1=dl[:, bi * K + k:bi * K + k + 1].to_broadcast(
                                    [128, 128]),
                                op=mm.is_equal)
                            msg = ck[:, :, 0:128].rearrange(
                                "p k (h c) -> p (k h) c", c=16)
                            exb = ck[:, :, 128:136].rearrange(
                                "p k (h o) -> p (k h) o", o=1).to_broadcast(
                                [128, 8, 16])
                            nc.vector.tensor_tensor(out=msg, in0=msg, in1=exb,
                                                    op=mm.mult)
                            nc.tensor.matmul(
                                ps[:],
                                lhsT=S[:],
                                rhs=ck[:, :, 0:136].rearrange("p k w -> p (k w)"),
                                start=(k == 0), stop=(k == K - 1))
                        # epilogue: h = psum[:, :128] / den + b1 ; elu
                        rd = sml.tile([128, 8], F32, tag="rd")
                        nc.vector.tensor_scalar_add(rd[:], ps[:, 128:136], EPS)
                        nc.vector.reciprocal(rd[:], rd[:])
                        hb = hbp.tile([128, 128], F32, tag="hb")
                        nc.vector.tensor_tensor(
                            out=hb[:].rearrange("p (h c) -> p h c", c=16),
                            in0=ps[:, 0:128].rearrange("p (h c) -> p h c", c=16),
                            in1=rd[:].rearrange("p (h o) -> p h o",
                                                o=1).to_broadcast([128, 8, 16]),
                            op=mm.mult)
                        nc.vector.tensor_tensor(out=hb[:], in0=hb[:], in1=B1t[:],
                                                op=mm.add)
                        tm = hbp.tile([128, 128], F32, tag="tm")
                        nc.vector.tensor_scalar_min(tm[:], hb[:], 0.0)
                        nc.scalar.activation(out=tm[:], in_=tm[:], func=ACT.Exp)
                        nc.vector.tensor_scalar_sub(tm[:], tm[:], 1.0)
                        nc.vector.tensor_tensor(out=hb[:], in0=hb[:], in1=tm[:],
                                                op=mm.max)
                        # fused L2 node stage for this block
                        n0 = (sb * G + bi) * BLK
                        pt = psa2p.tile([128, 128], F32, tag="pa2")
                        nc.tensor.transpose(pt[:, :BLK], hb[:BLK, :],
                                            IDENT[:BLK, :BLK])
                        hTb = a2p.tile([128, 128], F32, tag="hTb")
                        nc.vector.tensor_copy(out=hTb[:, :BLK], in_=pt[:, :BLK])
                        p2m = psa2p.tile([64, 128], F32, tag="pa2")
                        nc.tensor.matmul(p2m[:, :BLK], lhsT=W2t[:],
                                         rhs=hTb[:, :BLK],
                                         start=True, stop=True)
                        x2T = a2p.tile([64, 128], F32, tag="x2T")
                        nc.vector.tensor_copy(out=x2T[:, :BLK], in_=p2m[:, :BLK])
                        p2a = psa2p.tile([2, 128], F32, tag="pa2")
                        nc.tensor.matmul(p2a[:, :BLK], lhsT=AA2t[:],
                                         rhs=x2T[:, :BLK], start=True, stop=True)
                        a2T = a2p.tile([2, 128], F32, tag="a2T")
                        nc.vector.tensor_copy(out=a2T[:, :BLK], in_=p2a[:, :BLK])
                        p2t = psa2p.tile([128, 64], F32, tag="pa2")
                        nc.tensor.transpose(p2t[:BLK, :], x2T[:, :BLK],
                                            IDENT[:64, :64])
                        x2 = a2p.tile([128, 64], TD, tag="x2")
                        nc.vector.tensor_copy(out=x2[:BLK, :], in_=p2t[:BLK, :])
                        p2u = psa2p.tile([128, 2], F32, tag="pa2")
                        nc.tensor.transpose(p2u[:BLK, :], a2T[:, :BLK],
                                            IDENT[:2, :2])
                        a2 = a2p.tile([128, 64], TD, tag="a2")
                        nc.vector.memset(a2[:, 2:64], 0.0)
                        nc.vector.tensor_copy(out=a2[:BLK, :2], in_=p2u[:BLK, :])
                        nc.sync.dma_start(out=P2L[n0:n0 + BLK, 0:64],
                                          in_=x2[:BLK, :])
                        nc.sync.dma_start(out=P2L[n0:n0 + BLK, 64:128],
                                          in_=a2[:BLK, :])

            with nc.named_scope("ag2"):
                nc.gpsimd.collective_compute(
                    "AllGather", mm.bypass, replica_groups=RG,
                    ins=[P2L[:, :]], outs=[P2F[:, :]])

            # ---------------- L2 edge stage ----------------
            with nc.named_scope("edge2"), \
                 tc.tile_pool(name="gma2", bufs=2) as gmap2, \
                 tc.tile_pool(name="gmb2", bufs=2) as gmbp2, \
                 tc.tile_pool(name="gad2", bufs=2) as gadp2, \
                 tc.tile_pool(name="off2", bufs=2) as offp2, \
                 tc.tile_pool(name="sml2", bufs=3) as sml2, \
                 tc.tile_pool(name="sp2", bufs=3) as spp2, \
                 tc.tile_pool(name="ob", bufs=2) as obp, \
                 tc.tile_pool(name="ps2", bufs=2, space="PSUM") as ps2p:

                for sb in range(NSB):
                    ix = offp2.tile([128, CW], I16, tag="ix2")
                    nc.sync.dma_start(out=ix[:], in_=IDX[sb, :, :])
                    gmA = gmap2.tile([128, G * Ka * P2W], TD, tag="gmA2")
                    gmA3 = gmA[:].rearrange("p (r w) -> p r w", w=P2W)
                    gcalls(ix, gmA3, P2F[0:HALF, :], 0, G * Ka, P2W)
                    gmB = gmbp2.tile([128, G * Kb * P2W], TD, tag="gmB2")
                    gmB3 = gmB[:].rearrange("p (r w) -> p r w", w=P2W)
                    gcalls(ix, gmB3, P2F[HALF:n_nodes, :], G * Ka, G * Kb, P2W)
                    G2E = P2W if BF16 else 64
                    G2O = 0 if BF16 else 64
                    A2C = 65 if BF16 else 1
                    gad = gadp2.tile([128, G * K * G2E], TD, tag="gad2")
                    gad3 = gad[:].rearrange("p (r w) -> p r w", w=G2E)
                    gcalls(ix, gad3, P2L[:, G2O:G2O + G2E], G * (Ka + Kb), G * K,
                           G2E, estep=P2W)
                    dl = sml2.tile([128, G * K], TD, tag="dl2")
                    nc.sync.dma_start(out=dl[:], in_=DLOC[sb, :, :])

                    for bi in range(G):
                        asA = gmA3[:, bi * Ka:(bi + 1) * Ka, 64:65].rearrange(
                            "p k w -> p (k w)")
                        asB = gmB3[:, bi * Kb:(bi + 1) * Kb, 64:65].rearrange(
                            "p k w -> p (k w)")
                        ea = sml2.tile([128, K], F32, tag="ea2")
                        nc.vector.tensor_tensor(
                            out=ea[:, 0:Ka], in0=asA,
                            in1=gad3[:, bi * K:bi * K + Ka, A2C:A2C + 1].rearrange(
                                "p k w -> p (k w)"), op=mm.add)
                        nc.vector.tensor_tensor(
                            out=ea[:, Ka:K], in0=asB,
                            in1=gad3[:, bi * K + Ka:(bi + 1) * K, A2C:A2C + 1].rearrange(
                                "p k w -> p (k w)"), op=mm.add)
                        tl = sml2.tile([128, K], F32, tag="tl2")
                        nc.vector.tensor_scalar_mul(tl[:], ea[:], NEG_SLOPE)
                        nc.vector.tensor_tensor(out=ea[:], in0=ea[:], in1=tl[:],
                                                op=mm.max)
                        nc.scalar.activation(out=asA, in_=ea[:, 0:Ka],
                                             func=ACT.Exp)
                        nc.scalar.activation(out=asB, in_=ea[:, Ka:K],
                                             func=ACT.Exp)
                        ps = ps2p.tile([128, 65], F32, tag="psb")
                        for k in range(K):
                            if k < Ka:
                                ck = gmA3[:, bi * Ka + k:bi * Ka + k + 1, :]
                            else:
                                kk = bi * Kb + (k - Ka)
                                ck = gmB3[:, kk:kk + 1, :]
                            ckm = ck[:, :, 0:64].rearrange("p k w -> p (k w)")
                            nc.vector.tensor_tensor(
                                out=ckm, in0=ckm,
                                in1=ck[:, :, 64:65].rearrange(
                                    "p k w -> p (k w)").to_broadcast([128, 64]),
                                op=mm.mult)
                            S = spp2.tile([128, 128], TD, tag="S2")
                            nc.vector.tensor_tensor(
                                out=S[:], in0=IOTAt[:],
                                in1=dl[:, bi * K + k:bi * K + k + 1].to_broadcast(
                                    [128, 128]),
                                op=mm.is_equal)
                            nc.tensor.matmul(
                                ps[:],
                                lhsT=S[:],
                                rhs=ck[:, :, 0:65].rearrange("p k w -> p (k w)"),
                                start=(k == 0), stop=(k == K - 1))
                        rd = sml2.tile([128, 1], F32, tag="rd2")
                        nc.vector.tensor_scalar_add(rd[:], ps[:, 64:65], EPS)
                        nc.vector.reciprocal(rd[:], rd[:])
                        ob = obp.tile([128, 64], F32, tag="ob")
                        nc.vector.tensor_tensor(out=ob[:], in0=ps[:, 0:64],
                                                in1=rd[:].to_broadcast([128, 64]),
                                                op=mm.mult)
                        nc.vector.tensor_tensor(out=ob[:], in0=ob[:], in1=B2t[:],
                                                op=mm.add)
                        n0 = (sb * G + bi) * BLK
                        nc.sync.dma_start(out=OUT[n0:n0 + BLK, :], in_=ob[:BLK, :])

    nc.compile()
    return nc


def _run(inputs, sim=False, trace=False):
    in_maps, prm = _host_prep(**inputs)
    nc = _build_program(prm)
    n_cores = prm["n_cores"]
    if sim:
        from concourse.bass_interp import MultiCoreSim
        ms = MultiCoreSim(nc, num_cores=n_cores)
        for c in range(n_cores):
            for k, v in in_maps[c].items():
                ms.cores[c].tensor(k)[:] = v
        ms.simulate()
        outs = [np.array(ms.cores[c].tensor("out")) for c in range(n_cores)]
        got = np.concatenate(outs, axis=0)
    else:
        from concourse.bass_utils import run_bass_kernel_spmd
        res = run_bass_kernel_spmd(nc, in_maps, core_ids=list(range(n_cores)),
                                   trace=trace)
        outs = [res.results[c]["out"] for c in range(n_cores)]
        got = np.concatenate(outs, axis=0)
    full = np.empty_like(got)
    full[prm["perm"]] = got
    return (full, None) if sim else (full, res)


def kernel(**inputs):
    out, _ = _run({k: np.asarray(v) for k, v in inputs.items()})
    return out


# revision 25
# speedup vs baseline: 2.5645x; 2.5645x over previous
"""BiGAT (2-layer GAT, PyG-style with self-loops) on 8 Trainium2 NeuronCores.

Strategy: partition nodes (and their incoming edges) by destination across 8
cores. Nodes are permuted so every 125-node dst block carries a near-equal
edge count in both src-halves (two-pass balancing), making the per-block
chunk count uniform and small. Edges are sorted by dst and padded to a
uniform blocks-x-chunks structure so one SPMD program serves all cores.

The measured runtime cost structure on this stack is ~100us PER INSTRUCTION
(dispatch-bound; payload size nearly free), so the kernel minimizes the
instruction count:
  - gathers batched into maximal 1024-idx calls spanning G=5 blocks;
  - per-edge score/softmax vector work merged into a handful of whole-
    super-block instructions (strided multi-dim access patterns);
  - the one-hot scatter matrices for a block are built by a single
    is_equal; gathered rows are packed [xh | a_src | a_dst | pad] so the
    exp() output can overwrite a_src in place and each chunk's scatter is
    ONE PSUM-accumulating matmul over [msg | ex].

Per layer:
  node stage : xh = x @ W and attention dots (PE); packed rows written to a
               local DRAM table; AllGather replicates the table.
  edge stage : dma_gather rows by src and [a|..] rows by dst-local;
               e = lrelu(a_src+a_dst); ex = exp(e) (softmax max-shift
               skipped -- scores are O(10) so exp cannot overflow, and
               softmax is shift-invariant); msg = xh_src * ex; one-hot
               matmul scatter-adds [msg | ex] into PSUM per block; epilogue
               divides by the summed ex, adds bias. The L2 node stage is
               fused into the L1 edge epilogue.

dma_gather constraints honored: int16 indices (src tables split into two
<=25000-row halves; dst uses core-local indices), <=1024 idxs per call,
row strides and elem sizes multiples of 256B, indices wrapped [16, n/16]
and replicated to 128 partitions.
"""
import sys

sys.path.insert(0, "/opt/trn_rl_repo")

import heapq
import numpy as np

from concourse import bass, mybir
import concourse.bacc as bacc
import concourse.tile as tile
from concourse.masks import make_identity

F32 = mybir.dt.float32
I16 = mybir.dt.int16
TD = mybir.dt.bfloat16
import ml_dtypes
TNP = ml_dtypes.bfloat16

# ---------------- problem constants (hardcoded per contract) ----------------
N_NODES = 50000
N_EDGES = 800000
IN_C, HID_C, OUT_C, HEADS = 128, 16, 64, 8
NEG_SLOPE = 0.2
N_CORES = 8

# ---------------- sharding / tiling parameters ----------------
BLK = 125       # dst nodes per edge-stage block (<=128 for one-hot)
P1W = 256       # L1 table row: [xh(128) | a_src(8) | a_dst(8) | pad]
P2W = 128       # L2 table row: [x2(64) | a_src2(1) | a_dst2(1) | pad]
G = 5           # blocks per super-block (gather batching unit)
MAXI = 1024     # max idxs per dma_gather call (HW cap; >1024 crashes)
EPS = 1e-16


def _wrap16(idx):
    """[L] int array -> dma_gather wrapped layout [128, L//16] int16."""
    L = len(idx)
    w = idx.reshape(L // 16, 16).T
    return np.tile(w, (8, 1)).astype(np.int16)


def _balance_blocks2(src0, dst0, n_nodes, nblk_tot):
    """Two-pass node->block assignment: pass 1 equalizes total in-degree
    (fixing each node's half); pass 2 reassigns within each half to equalize
    per-block edge counts from BOTH halves. Returns perm (new pos -> node)."""
    HALF = n_nodes // 2
    deg = np.bincount(dst0, minlength=n_nodes)
    order = np.argsort(-deg, kind="stable")
    fill = np.zeros(nblk_tot, np.int32)
    perm = np.empty(n_nodes, np.int64)
    heap = [(0, b) for b in range(nblk_tot)]
    heapq.heapify(heap)
    for node in order:
        load, b = heapq.heappop(heap)
        perm[b * BLK + fill[b]] = node
        fill[b] += 1
        load += int(deg[node])
        if fill[b] < BLK:
            heapq.heappush(heap, (load, b))

    old2new = np.empty(n_nodes, np.int64)
    old2new[perm] = np.arange(n_nodes)
    src_in_a = old2new[src0] < HALF
    deg_a = np.bincount(dst0, weights=src_in_a.astype(np.float64),
                        minlength=n_nodes).astype(np.int64)
    deg_b = deg - deg_a

    perm2 = np.empty(n_nodes, np.int64)
    nb2 = nblk_tot // 2
    for half in (0, 1):
        nodes = perm[half * HALF:(half + 1) * HALF]
        o = np.argsort(-(deg_a[nodes] + deg_b[nodes]), kind="stable")
        nodes = nodes[o]
        la = np.zeros(nb2, np.int64)
        lb = np.zeros(nb2, np.int64)
        fl = np.zeros(nb2, np.int32)
        for node in nodes:
            da, db = deg_a[node], deg_b[node]
            cand = np.maximum(la + da, lb + db).astype(np.float64)
            cand[fl >= BLK] = np.inf
            b = int(np.argmin(cand))
            perm2[half * HALF + b * BLK + fl[b]] = node
            fl[b] += 1
            la[b] += da
            lb[b] += db
    return perm2


def _host_prep(x, edge_index, W1, att_src1, att_dst1, b1, W2, att_src2, att_dst2, b2,
               n_nodes=N_NODES, n_cores=N_CORES):
    """Sort/pad edges, build per-core input maps and compile-time params."""
    NP = n_nodes // n_cores
    NB = NP // BLK
    assert NB * BLK == NP and NB % G == 0
    NSB = NB // G
    HALF = n_nodes // 2
    assert HALF < 32768 and NP < 32768

    src0 = np.concatenate([np.asarray(edge_index[0]), np.arange(n_nodes)]).astype(np.int64)
    dst0 = np.concatenate([np.asarray(edge_index[1]), np.arange(n_nodes)]).astype(np.int64)

    nblk_tot = n_cores * NB
    perm = _balance_blocks2(src0, dst0, n_nodes, nblk_tot)  # new pos -> old node
    old2new = np.empty(n_nodes, np.int64)
    old2new[perm] = np.arange(n_nodes)

    src = old2new[src0]
    dst = old2new[dst0]
    order = np.argsort(dst, kind="stable")
    src, dst = src[order], dst[order]

    blk_of = dst // BLK
    # within each dst-block, put src<HALF ("a") edges first
    order2 = np.lexsort((src >= HALF, blk_of))
    src, dst = src[order2], dst[order2]
    is_b = src >= HALF
    cnt_a = np.bincount(blk_of[order2], weights=~is_b, minlength=nblk_tot).astype(np.int64)
    cnt_b = np.bincount(blk_of[order2], weights=is_b, minlength=nblk_tot).astype(np.int64)
    starts = np.concatenate([[0], np.cumsum(cnt_a + cnt_b)]).astype(np.int64)
    Ka = int(np.ceil(cnt_a.max() / 128))
    Kb = int(np.ceil(cnt_b.max() / 128))
    K = Ka + Kb

    # per-block padded arrays in [a-pad | b-pad] chunk order
    srcA = np.zeros((nblk_tot, Ka * 128), np.int64)      # pad -> row 0
    srcB = np.zeros((nblk_tot, Kb * 128), np.int64)
    dstL = np.zeros((nblk_tot, K * 128), np.int64)       # dst local to core
    dloc = np.full((nblk_tot, K * 128), 999.0, np.float32)  # dst local to block
    for b in range(nblk_tot):
        na, nb_ = int(cnt_a[b]), int(cnt_b[b])
        s = starts[b]
        core = b // NB
        srcA[b, :na] = src[s:s + na]
        srcB[b, :nb_] = src[s + na:s + na + nb_] - HALF
        dstL[b, :na] = dst[s:s + na] - core * NP
        dstL[b, Ka * 128:Ka * 128 + nb_] = dst[s + na:s + na + nb_] - core * NP
        dloc[b, :na] = dst[s:s + na] - b * BLK
        dloc[b, Ka * 128:Ka * 128 + nb_] = dst[s + na:s + na + nb_] - b * BLK

    # shared (replicated) weights
    AA1 = np.zeros((128, 16), np.float32)
    asrc1 = np.asarray(att_src1, np.float32)
    adst1 = np.asarray(att_dst1, np.float32)
    for h in range(HEADS):
        AA1[16 * h:16 * (h + 1), h] = asrc1[h]
        AA1[16 * h:16 * (h + 1), 8 + h] = adst1[h]
    AA2 = np.stack([np.asarray(att_src2, np.float32)[0],
                    np.asarray(att_dst2, np.float32)[0]], axis=1)  # [64, 2]
    shared = {
        "W1": np.asarray(W1, np.float32),
        "AA1": AA1,
        "B1": np.tile(np.asarray(b1, np.float32), (128, 1)),
        "W2": np.asarray(W2, np.float32),
        "AA2": AA2,
        "B2": np.tile(np.asarray(b2, np.float32), (128, 1)),
        "IOTA": np.tile(np.arange(128), (128, 1)).astype(TNP),
    }

    xT = np.ascontiguousarray(np.asarray(x, np.float32).T)  # [128, N] (old order)

    in_maps = []
    for c in range(n_cores):
        lo = c * NB
        # super-block idx layout: [A(b0..) | B(b0..) | dst(b0..)] wrapped
        idx = np.stack([
            np.concatenate(
                [_wrap16(srcA[lo + s * G + g]) for g in range(G)] +
                [_wrap16(srcB[lo + s * G + g]) for g in range(G)] +
                [_wrap16(dstL[lo + s * G + g]) for g in range(G)], axis=1)
            for s in range(NSB)])
        dl = np.stack([
            np.concatenate(
                [dloc[lo + s * G + g].reshape(K, 128).T for g in range(G)], axis=1)
            for s in range(NSB)])                        # [NSB, 128, G*K]
        m = dict(shared)
        m["xT"] = np.ascontiguousarray(xT[:, perm[c * NP:(c + 1) * NP]])
        m["IDX"] = np.ascontiguousarray(idx)
        m["DLOC"] = np.ascontiguousarray(dl.astype(TNP))
        in_maps.append(m)

    prm = dict(NP=NP, NB=NB, NSB=NSB, K=K, Ka=Ka, Kb=Kb,
               n_nodes=n_nodes, n_cores=n_cores, HALF=HALF, perm=perm)
    return in_maps, prm


def _build_program(prm, repeat=1):
    import os
    SKIP_AG = bool(int(os.environ.get("BG_SKIP_AG", "0")))
    SKIP_EDGE = bool(int(os.environ.get("BG_SKIP_EDGE", "0")))
    SKIP_NODE = bool(int(os.environ.get("BG_SKIP_NODE", "0")))
    NP, NSB, K, Ka, Kb = prm["NP"], prm["NSB"], prm["K"], prm["Ka"], prm["Kb"]
    HALF = prm["HALF"]
    n_nodes, n_cores = prm["n_nodes"], prm["n_cores"]
    RG = [list(range(n_cores))]
    CW = G * (Ka + Kb + K) * 8  # idx tensor cols per super-block

    nc = bacc.Bacc("TRN2", target_bir_lowering=False, debug=False,
                   num_devices=n_cores, num_swdge_queues=4)
    qn = [0]  # round-robin SWDGE queue assignment for gathers

    def next_q():
        qn[0] += 1
        return qn[0] % 4

    # inputs
    xT = nc.dram_tensor("xT", [128, NP], F32, kind="ExternalInput")
    W1 = nc.dram_tensor("W1", [128, 128], F32, kind="ExternalInput")
    AA1 = nc.dram_tensor("AA1", [128, 16], F32, kind="ExternalInput")
    B1 = nc.dram_tensor("B1", [128, 128], F32, kind="ExternalInput")
    W2 = nc.dram_tensor("W2", [128, 64], F32, kind="ExternalInput")
    AA2 = nc.dram_tensor("AA2", [64, 2], F32, kind="ExternalInput")
    B2 = nc.dram_tensor("B2", [128, 64], F32, kind="ExternalInput")
    IOTA = nc.dram_tensor("IOTA", [128, 128], TD, kind="ExternalInput")
    IDX = nc.dram_tensor("IDX", [NSB, 128, CW], I16, kind="ExternalInput")
    DLOC = nc.dram_tensor("DLOC", [NSB, 128, G * K], TD, kind="ExternalInput")
    OUT = nc.dram_tensor("out", [NP, OUT_C], F32, kind="ExternalOutput")
    # internal DRAM
    P1L = nc.dram_tensor("P1L", [NP, P1W], TD)
    P1F = nc.dram_tensor("P1F", [n_nodes, P1W], TD, addr_space="Shared")
    P2L = nc.dram_tensor("P2L", [NP, P2W], TD)
    P2F = nc.dram_tensor("P2F", [n_nodes, P2W], TD, addr_space="Shared")

    mm = mybir.AluOpType
    ACT = mybir.ActivationFunctionType

    def gcalls(ix, out3, table, col0, nchunk, elem, estep=None):
        step = MAXI // 128
        for c0 in range(0, nchunk, step):
            c1 = min(c0 + step, nchunk)
            nc.gpsimd.dma_gather(
                out_ap=out3[:, c0:c1, :], in_ap=table,
                idxs_ap=ix[:, (col0 + c0) * 8:(col0 + c1) * 8],
                num_idxs=(c1 - c0) * 128,
                num_idxs_reg=(c1 - c0) * 128, elem_size=elem,
                elem_step=estep, queue_num=next_q())

    from contextlib import ExitStack
    with tile.TileContext(nc) as tc, ExitStack() as ctx:
        cst = ctx.enter_context(tc.tile_pool(name="cst", bufs=1))
        W1t = cst.tile([128, 128], F32)
        AA1t = cst.tile([128, 16], F32)
        B1t = cst.tile([128, 128], F32)
        W2t = cst.tile([128, 64], F32)
        AA2t = cst.tile([64, 2], F32)
        B2t = cst.tile([128, 64], F32)
        IOTAt = cst.tile([128, 128], TD)
        IDENT = cst.tile([128, 128], F32)
        for t, d in ((W1t, W1), (AA1t, AA1), (B1t, B1), (W2t, W2),
                     (AA2t, AA2), (B2t, B2), (IOTAt, IOTA)):
            nc.sync.dma_start(out=t[:], in_=d[:, :])
        make_identity(nc, IDENT[:])

        # body may be repeated for differential benchmarking
        for _rep in range(repeat):
            # ---------------- stage A: L1 node stage ----------------
            with nc.named_scope("nodeA"), \
                 tc.tile_pool(name="pa", bufs=3) as pa, \
                 tc.tile_pool(name="ppa", bufs=3, space="PSUM") as ppa:
                for c0 in ([] if SKIP_NODE else range(0, NP, 128)):
                    nn = min(128, NP - c0)
                    xt = pa.tile([128, 128], F32, tag="xt")
                    nc.sync.dma_start(out=xt[:, :nn], in_=xT[:, c0:c0 + nn])
                    pm = ppa.tile([128, 128], F32, tag="pp")
                    nc.tensor.matmul(pm[:, :nn], lhsT=W1t[:], rhs=xt[:, :nn],
                                     start=True, stop=True)
                    xhT = pa.tile([128, 128], F32, tag="xhT")
                    nc.vector.tensor_copy(out=xhT[:, :nn], in_=pm[:, :nn])
                    pm2 = ppa.tile([16, 128], F32, tag="pp")
                    nc.tensor.matmul(pm2[:, :nn], lhsT=AA1t[:], rhs=xhT[:, :nn],
                                     start=True, stop=True)
                    aaT = pa.tile([16, 128], F32, tag="aaT")
                    nc.vector.tensor_copy(out=aaT[:, :nn], in_=pm2[:, :nn])
                    pt = ppa.tile([128, 128], F32, tag="pp")
                    nc.tensor.transpose(pt[:nn, :], xhT[:, :nn], IDENT[:])
                    xh = pa.tile([128, 128], TD, tag="xh")
                    nc.vector.tensor_copy(out=xh[:nn, :], in_=pt[:nn, :])
                    pt2 = ppa.tile([128, 16], F32, tag="pp")
                    nc.tensor.transpose(pt2[:nn, :], aaT[:, :nn], IDENT[:16, :16])
                    aa = pa.tile([128, P1W - 128], TD, tag="aa")
                    nc.vector.memset(aa[:, 16:], 0.0)
                    nc.vector.tensor_copy(out=aa[:nn, :16], in_=pt2[:nn, :])
                    nc.sync.dma_start(out=P1L[c0:c0 + nn, 0:128], in_=xh[:nn, :])
                    nc.sync.dma_start(out=P1L[c0:c0 + nn, 128:P1W], in_=aa[:nn, :])

            with nc.named_scope("ag1"):
                if not SKIP_AG:
                    nc.gpsimd.collective_compute(
                        "AllGather", mm.bypass, replica_groups=RG,
                        ins=[P1L[:, :]], outs=[P1F[:, :]])

            # ---------------- L1 edge stage (+ fused L2 node stage) ----------------
            with nc.named_scope("edge1"), \
                 tc.tile_pool(name="gma", bufs=2) as gmap, \
                 tc.tile_pool(name="gmb", bufs=2) as gmbp, \
                 tc.tile_pool(name="gad", bufs=2) as gadp, \
                 tc.tile_pool(name="off", bufs=2) as offp, \
                 tc.tile_pool(name="sml", bufs=3) as sml, \
                 tc.tile_pool(name="sal", bufs=2) as salp, \
                 tc.tile_pool(name="hb", bufs=2) as hbp, \
                 tc.tile_pool(name="a2", bufs=2) as a2p, \
                 tc.tile_pool(name="ps1", bufs=2, space="PSUM") as ps1p, \
                 tc.tile_pool(name="psa2", bufs=2, space="PSUM") as psa2p:

                for sb in ([] if SKIP_EDGE else range(NSB)):
                    ix = offp.tile([128, CW], I16, tag="ix")
                    nc.sync.dma_start(out=ix[:], in_=IDX[sb, :, :])
                    gmA = gmap.tile([128, G * Ka * P1W], TD, tag="gmA")
                    gmA3 = gmA[:].rearrange("p (r w) -> p r w", w=P1W)
                    gmA4 = gmA[:].rearrange("p (g k w) -> p g k w", g=G, w=P1W)
                    gcalls(ix, gmA3, P1F[0:HALF, :], 0, G * Ka, P1W)
                    gmB = gmbp.tile([128, G * Kb * P1W], TD, tag="gmB")
                    gmB3 = gmB[:].rearrange("p (r w) -> p r w", w=P1W)
                    gmB4 = gmB[:].rearrange("p (g k w) -> p g k w", g=G, w=P1W)
                    gcalls(ix, gmB3, P1F[HALF:n_nodes, :], G * Ka, G * Kb, P1W)
                    gad = gadp.tile([128, G * K * (P1W - 128)], TD, tag="gad")
                    gad3 = gad[:].rearrange("p (r w) -> p r w", w=P1W - 128)
                    gad4 = gad[:].rearrange("p (g k w) -> p g k w", g=G,
                                            w=P1W - 128)
                    gcalls(ix, gad3, P1L[:, 128:P1W], G * (Ka + Kb), G * K,
                           P1W - 128, estep=P1W)
                    dl = sml.tile([128, G * K], TD, tag="dl")
                    nc.sync.dma_start(out=dl[:], in_=DLOC[sb, :, :])

                    # whole-super-block score pipeline (a few fat instructions)
                    eaA = sml.tile([128, G * Ka * 8], F32, tag="eaA")
                    eaA4 = eaA[:].rearrange("p (g k h) -> p g k h", g=G, h=8)
                    nc.vector.tensor_tensor(
                        out=eaA4, in0=gmA4[:, :, :, 128:136],
                        in1=gad4[:, :, 0:Ka, 8:16], op=mm.add)
                    eaB = sml.tile([128, G * Kb * 8], F32, tag="eaB")
                    eaB4 = eaB[:].rearrange("p (g k h) -> p g k h", g=G, h=8)
                    nc.vector.tensor_tensor(
                        out=eaB4, in0=gmB4[:, :, :, 128:136],
                        in1=gad4[:, :, Ka:K, 8:16], op=mm.add)
                    tlA = sml.tile([128, G * Ka * 8], F32, tag="tlA")
                    nc.vector.tensor_scalar_mul(tlA[:], eaA[:], NEG_SLOPE)
                    nc.vector.tensor_tensor(out=eaA[:], in0=eaA[:], in1=tlA[:],
                                            op=mm.max)
                    tlB = sml.tile([128, G * Kb * 8], F32, tag="tlB")
                    nc.vector.tensor_scalar_mul(tlB[:], eaB[:], NEG_SLOPE)
                    nc.vector.tensor_tensor(out=eaB[:], in0=eaB[:], in1=tlB[:],
                                            op=mm.max)
                    nc.scalar.activation(out=gmA4[:, :, :, 128:136], in_=eaA4,
                                         func=ACT.Exp)
                    nc.scalar.activation(out=gmB4[:, :, :, 128:136], in_=eaB4,
                                         func=ACT.Exp)
                    # msg = xh * ex, whole super-block per half
                    nc.vector.tensor_tensor(
                        out=gmA3[:, :, 0:128].rearrange(
                            "p r (h c) -> p r h c", c=16),
                        in0=gmA3[:, :, 0:128].rearrange(
                            "p r (h c) -> p r h c", c=16),
                        in1=gmA3[:, :, 128:136].rearrange(
                            "p r (h o) -> p r h o", o=1).to_broadcast(
                            [128, G * Ka, 8, 16]),
                        op=mm.mult)
                    nc.vector.tensor_tensor(
                        out=gmB3[:, :, 0:128].rearrange(
                            "p r (h c) -> p r h c", c=16),
                        in0=gmB3[:, :, 0:128].rearrange(
                            "p r (h c) -> p r h c", c=16),
                        in1=gmB3[:, :, 128:136].rearrange(
                            "p r (h o) -> p r h o", o=1).to_broadcast(
                            [128, G * Kb, 8, 16]),
                        op=mm.mult)

                    for bi in range(G):
                        n0 = (sb * G + bi) * BLK
                        S_all = salp.tile([128, K * 128], TD, tag="sall")
                        nc.vector.tensor_tensor(
                            out=S_all[:].rearrange("p (k e) -> p k e", e=128),
                            in0=IOTAt[:].rearrange(
                                "p (o e) -> p o e", o=1).to_broadcast(
                                [128, K, 128]),
                            in1=dl[:, bi * K:(bi + 1) * K].rearrange(
                                "p (k o) -> p k o", o=1).to_broadcast(
                                [128, K, 128]),
                            op=mm.is_equal)
                        ps = ps1p.tile([128, 136], F32, tag="ps")
                        for k in range(K):
                            if k < Ka:
                                r = bi * Ka + k
                                ck = gmA[:, r * P1W:r * P1W + 136]
                            else:
                                r = bi * Kb + (k - Ka)
                                ck = gmB[:, r * P1W:r * P1W + 136]
                            nc.tensor.matmul(
                                ps[:], lhsT=S_all[:, k * 128:(k + 1) * 128],
                                rhs=ck, start=(k == 0), stop=(k == K - 1))
                        # epilogue: h = psum[:, :128] / den + b1 ; elu
                        rd = sml.tile([128, 8], F32, tag="rd")
                        nc.vector.tensor_scalar_add(rd[:], ps[:, 128:136], EPS)
                        nc.vector.reciprocal(rd[:], rd[:])
                        hb = hbp.tile([128, 128], F32, tag="hb")
                        nc.vector.tensor_tensor(
                            out=hb[:].rearrange("p (h c) -> p h c", c=16),
                            in0=ps[:, 0:128].rearrange("p (h c) -> p h c", c=16),
                            in1=rd[:].rearrange("p (h o) -> p h o",
                                                o=1).to_broadcast([128, 8, 16]),
                            op=mm.mult)
                        nc.vector.tensor_tensor(out=hb[:], in0=hb[:], in1=B1t[:],
                                                op=mm.add)
                        tm = hbp.tile([128, 128], F32, tag="tm")
                        nc.vector.tensor_scalar_min(tm[:], hb[:], 0.0)
                        nc.scalar.activation(out=tm[:], in_=tm[:], func=ACT.Exp)
                        nc.vector.tensor_scalar_sub(tm[:], tm[:], 1.0)
                        nc.vector.tensor_tensor(out=hb[:], in0=hb[:], in1=tm[:],
                                                op=mm.max)
                        # fused L2 node stage for this block
                        pt = psa2p.tile([128, 128], F32, tag="pa2")
                        nc.tensor.transpose(pt[:, :BLK], hb[:BLK, :],
                                            IDENT[:BLK, :BLK])
                        hTb = a2p.tile([128, 128], F32, tag="hTb")
                        nc.vector.tensor_copy(out=hTb[:, :BLK], in_=pt[:, :BLK])
                        p2m = psa2p.tile([64, 128], F32, tag="pa2")
                        nc.tensor.matmul(p2m[:, :BLK], lhsT=W2t[:],
                                         rhs=hTb[:, :BLK],
                                         start=True, stop=True)
                        x2T = a2p.tile([64, 128], F32, tag="x2T")
                        nc.vector.tensor_copy(out=x2T[:, :BLK], in_=p2m[:, :BLK])
                        p2a = psa2p.tile([2, 128], F32, tag="pa2")
                        nc.tensor.matmul(p2a[:, :BLK], lhsT=AA2t[:],
                                         rhs=x2T[:, :BLK], start=True, stop=True)
                        a2T = a2p.tile([2, 128], F32, tag="a2T")
                        nc.vector.tensor_copy(out=a2T[:, :BLK], in_=p2a[:, :BLK])
                        p2t = psa2p.tile([128, 64], F32, tag="pa2")
                        nc.tensor.transpose(p2t[:BLK, :], x2T[:, :BLK],
                                            IDENT[:64, :64])
                        x2 = a2p.tile([128, 64], TD, tag="x2")
                        nc.vector.tensor_copy(out=x2[:BLK, :], in_=p2t[:BLK, :])
                        p2u = psa2p.tile([128, 2], F32, tag="pa2")
                        nc.tensor.transpose(p2u[:BLK, :], a2T[:, :BLK],
                                            IDENT[:2, :2])
                        a2 = a2p.tile([128, 64], TD, tag="a2")
                        nc.vector.memset(a2[:, 2:64], 0.0)
                        nc.vector.tensor_copy(out=a2[:BLK, :2], in_=p2u[:BLK, :])
                        nc.sync.dma_start(out=P2L[n0:n0 + BLK, 0:64],
                                          in_=x2[:BLK, :])
                        nc.sync.dma_start(out=P2L[n0:n0 + BLK, 64:128],
                                          in_=a2[:BLK, :])

            with nc.named_scope("ag2"):
                if not SKIP_AG:
                    nc.gpsimd.collective_compute(
                        "AllGather", mm.bypass, replica_groups=RG,
                        ins=[P2L[:, :]], outs=[P2F[:, :]])

            # ---------------- L2 edge stage ----------------
            with nc.named_scope("edge2"), \
                 tc.tile_pool(name="gma2", bufs=2) as gmap2, \
                 tc.tile_pool(name="gmb2", bufs=2) as gmbp2, \
                 tc.tile_pool(name="gad2", bufs=2) as gadp2, \
                 tc.tile_pool(name="off2", bufs=2) as offp2, \
                 tc.tile_pool(name="sml2", bufs=3) as sml2, \
                 tc.tile_pool(name="sal2", bufs=2) as salp2, \
                 tc.tile_pool(name="ob", bufs=2) as obp, \
                 tc.tile_pool(name="ps2", bufs=2, space="PSUM") as ps2p:

                for sb in ([] if SKIP_EDGE else range(NSB)):
                    ix = offp2.tile([128, CW], I16, tag="ix2")
                    nc.sync.dma_start(out=ix[:], in_=IDX[sb, :, :])
                    gmA = gmap2.tile([128, G * Ka * P2W], TD, tag="gmA2")
                    gmA3 = gmA[:].rearrange("p (r w) -> p r w", w=P2W)
                    gmA4 = gmA[:].rearrange("p (g k w) -> p g k w", g=G, w=P2W)
                    gcalls(ix, gmA3, P2F[0:HALF, :], 0, G * Ka, P2W)
                    gmB = gmbp2.tile([128, G * Kb * P2W], TD, tag="gmB2")
                    gmB3 = gmB[:].rearrange("p (r w) -> p r w", w=P2W)
                    gmB4 = gmB[:].rearrange("p (g k w) -> p g k w", g=G, w=P2W)
                    gcalls(ix, gmB3, P2F[HALF:n_nodes, :], G * Ka, G * Kb, P2W)
                    gad = gadp2.tile([128, G * K * P2W], TD, tag="gad2")
                    gad3 = gad[:].rearrange("p (r w) -> p r w", w=P2W)
                    gad4 = gad[:].rearrange("p (g k w) -> p g k w", g=G, w=P2W)
                    gcalls(ix, gad3, P2L[:, :], G * (Ka + Kb), G * K, P2W)
                    dl = sml2.tile([128, G * K], TD, tag="dl2")
                    nc.sync.dma_start(out=dl[:], in_=DLOC[sb, :, :])

                    eaA = sml2.tile([128, G * Ka], F32, tag="eaA2")
                    eaA3 = eaA[:].rearrange("p (g k) -> p g k", g=G)
                    nc.vector.tensor_tensor(
                        out=eaA3,
                        in0=gmA4[:, :, :, 64:65].rearrange("p g k o -> p g (k o)"),
                        in1=gad4[:, :, 0:Ka, 65:66].rearrange("p g k o -> p g (k o)"),
                        op=mm.add)
                    eaB = sml2.tile([128, G * Kb], F32, tag="eaB2")
                    eaB3 = eaB[:].rearrange("p (g k) -> p g k", g=G)
                    nc.vector.tensor_tensor(
                        out=eaB3,
                        in0=gmB4[:, :, :, 64:65].rearrange("p g k o -> p g (k o)"),
                        in1=gad4[:, :, Ka:K, 65:66].rearrange("p g k o -> p g (k o)"),
                        op=mm.add)
                    tlA = sml2.tile([128, G * Ka], F32, tag="tlA2")
                    nc.vector.tensor_scalar_mul(tlA[:], eaA[:], NEG_SLOPE)
                    nc.vector.tensor_tensor(out=eaA[:], in0=eaA[:], in1=tlA[:],
                                            op=mm.max)
                    tlB = sml2.tile([128, G * Kb], F32, tag="tlB2")
                    nc.vector.tensor_scalar_mul(tlB[:], eaB[:], NEG_SLOPE)
                    nc.vector.tensor_tensor(out=eaB[:], in0=eaB[:], in1=tlB[:],
                                            op=mm.max)
                    nc.scalar.activation(
                        out=gmA4[:, :, :, 64:65].rearrange("p g k o -> p g (k o)"),
                        in_=eaA3, func=ACT.Exp)
                    nc.scalar.activation(
                        out=gmB4[:, :, :, 64:65].rearrange("p g k o -> p g (k o)"),
                        in_=eaB3, func=ACT.Exp)
                    nc.vector.tensor_tensor(
                        out=gmA3[:, :, 0:64], in0=gmA3[:, :, 0:64],
                        in1=gmA3[:, :, 64:65].to_broadcast([128, G * Ka, 64]),
                        op=mm.mult)
                    nc.vector.tensor_tensor(
                        out=gmB3[:, :, 0:64], in0=gmB3[:, :, 0:64],
                        in1=gmB3[:, :, 64:65].to_broadcast([128, G * Kb, 64]),
                        op=mm.mult)

                    for bi in range(G):
                        n0 = (sb * G + bi) * BLK
                        S_all = salp2.tile([128, K * 128], TD, tag="sall2")
                        nc.vector.tensor_tensor(
                            out=S_all[:].rearrange("p (k e) -> p k e", e=128),
                            in0=IOTAt[:].rearrange(
                                "p (o e) -> p o e", o=1).to_broadcast(
                                [128, K, 128]),
                            in1=dl[:, bi * K:(bi + 1) * K].rearrange(
                                "p (k o) -> p k o", o=1).to_broadcast(
                                [128, K, 128]),
                            op=mm.is_equal)
                        ps = ps2p.tile([128, 65], F32, tag="psb")
                        for k in range(K):
                            if k < Ka:
                                r = bi * Ka + k
                                ck = gmA[:, r * P2W:r * P2W + 65]
                            else:
                                r = bi * Kb + (k - Ka)
                                ck = gmB[:, r * P2W:r * P2W + 65]
                            nc.tensor.matmul(
                                ps[:], lhsT=S_all[:, k * 128:(k + 1) * 128],
                                rhs=ck, start=(k == 0), stop=(k == K - 1))
                        rd = sml2.tile([128, 1], F32, tag="rd2")
                        nc.vector.tensor_scalar_add(rd[:], ps[:, 64:65], EPS)
                        nc.vector.reciprocal(rd[:], rd[:])
                        ob = obp.tile([128, 64], F32, tag="ob")
                        nc.vector.tensor_tensor(out=ob[:], in0=ps[:, 0:64],
                                                in1=rd[:].to_broadcast([128, 64]),
                                                op=mm.mult)
                        nc.vector.tensor_tensor(out=ob[:], in0=ob[:], in1=B2t[:],
                                                op=mm.add)
                        nc.sync.dma_start(out=OUT[n0:n0 + BLK, :], in_=ob[:BLK, :])

    nc.compile()
    return nc


def _run(inputs, sim=False, trace=False):
    in_maps, prm = _host_prep(**inputs)
    nc = _build_program(prm)
    n_cores = prm["n_cores"]
    if sim:
        from concourse.bass_interp import MultiCoreSim
        ms = MultiCoreSim(nc, num_cores=n_cores)
        for c in range(n_cores):
            for k, v in in_maps[c].items():
                ms.cores[c].tensor(k)[:] = v
        ms.simulate()
        outs = [np.array(ms.cores[c].tensor("out")) for c in range(n_cores)]
        got = np.concatenate(outs, axis=0)
        res = None
    else:
        from concourse.bass_utils import run_bass_kernel_spmd
        res = run_bass_kernel_spmd(nc, in_maps, core_ids=list(range(n_cores)),
                                   trace=trace)
        outs = [res.results[c]["out"] for c in range(n_cores)]
        got = np.concatenate(outs, axis=0)
    full = np.empty_like(got)
    full[prm["perm"]] = got
    return full, res


def kernel(**inputs):
    out, _ = _run({k: np.asarray(v) for k, v in inputs.items()})
    return out


# revision 38
# speedup vs baseline: 3.2518x; 1.2680x over previous
"""BiGAT (2-layer GAT, PyG-style with self-loops) on 8 Trainium2 NeuronCores.

Strategy: partition nodes (and their incoming edges) by destination across 8
cores. Nodes are permuted so every 125-node dst block carries a near-equal
edge count in both src-halves (two-pass balancing), making the per-block
chunk count uniform and small. Edges are sorted by dst and padded to a
uniform blocks-x-chunks structure so one SPMD program serves all cores.

The measured runtime cost structure on this stack is ~100us PER INSTRUCTION
(dispatch-bound; payload size nearly free), so the kernel minimizes the
instruction count:
  - gathers batched into maximal 1024-idx calls spanning G=5 blocks;
  - per-edge score/softmax vector work merged into a handful of whole-
    super-block instructions (strided multi-dim access patterns);
  - the one-hot scatter matrices for a block are built by a single
    is_equal; gathered rows are packed [xh | a_src | a_dst | pad] so the
    exp() output can overwrite a_src in place and each chunk's scatter is
    ONE PSUM-accumulating matmul over [msg | ex].

Per layer:
  node stage : xh = x @ W and attention dots (PE); packed rows written to a
               local DRAM table; AllGather replicates the table.
  edge stage : dma_gather rows by src and [a|..] rows by dst-local;
               e = lrelu(a_src+a_dst); ex = exp(e) (softmax max-shift
               skipped -- scores are O(10) so exp cannot overflow, and
               softmax is shift-invariant); msg = xh_src * ex; one-hot
               matmul scatter-adds [msg | ex] into PSUM per block; epilogue
               divides by the summed ex, adds bias. The L2 node stage is
               fused into the L1 edge epilogue.

dma_gather constraints honored: int16 indices (src tables split into two
<=25000-row halves; dst uses core-local indices), <=1024 idxs per call,
row strides and elem sizes multiples of 256B, indices wrapped [16, n/16]
and replicated to 128 partitions.
"""
import sys

sys.path.insert(0, "/opt/trn_rl_repo")

import heapq
import numpy as np

from concourse import bass, mybir
import concourse.bacc as bacc
import concourse.tile as tile
from concourse.masks import make_identity

F32 = mybir.dt.float32
I16 = mybir.dt.int16
TD = mybir.dt.bfloat16
import ml_dtypes
TNP = ml_dtypes.bfloat16

# ---------------- problem constants (hardcoded per contract) ----------------
N_NODES = 50000
N_EDGES = 800000
IN_C, HID_C, OUT_C, HEADS = 128, 16, 64, 8
NEG_SLOPE = 0.2
N_CORES = 8

# ---------------- sharding / tiling parameters ----------------
BLK = 125       # dst nodes per edge-stage block (<=128 for one-hot)
P1W = 256       # L1 table row: [xh(128) | a_src(8) | a_dst(8) | pad]
P2W = 128       # L2 table row: [x2(64) | a_src2(1) | a_dst2(1) | pad]
G = 5           # blocks per super-block (gather batching unit)
MAXI = 1024     # max idxs per dma_gather call (HW cap; >1024 crashes)
EPS = 1e-16


def _wrap16(idx):
    """[L] int array -> dma_gather wrapped layout [128, L//16] int16."""
    L = len(idx)
    w = idx.reshape(L // 16, 16).T
    return np.tile(w, (8, 1)).astype(np.int16)


def _balance_blocks2(src0, dst0, n_nodes, nblk_tot):
    """Two-pass node->block assignment: pass 1 equalizes total in-degree
    (fixing each node's half); pass 2 reassigns within each half to equalize
    per-block edge counts from BOTH halves. Returns perm (new pos -> node)."""
    HALF = n_nodes // 2
    deg = np.bincount(dst0, minlength=n_nodes)
    order = np.argsort(-deg, kind="stable")
    fill = np.zeros(nblk_tot, np.int32)
    perm = np.empty(n_nodes, np.int64)
    heap = [(0, b) for b in range(nblk_tot)]
    heapq.heapify(heap)
    for node in order:
        load, b = heapq.heappop(heap)
        perm[b * BLK + fill[b]] = node
        fill[b] += 1
        load += int(deg[node])
        if fill[b] < BLK:
            heapq.heappush(heap, (load, b))

    old2new = np.empty(n_nodes, np.int64)
    old2new[perm] = np.arange(n_nodes)
    src_in_a = old2new[src0] < HALF
    deg_a = np.bincount(dst0, weights=src_in_a.astype(np.float64),
                        minlength=n_nodes).astype(np.int64)
    deg_b = deg - deg_a

    perm2 = np.empty(n_nodes, np.int64)
    nb2 = nblk_tot // 2
    for half in (0, 1):
        nodes = perm[half * HALF:(half + 1) * HALF]
        o = np.argsort(-(deg_a[nodes] + deg_b[nodes]), kind="stable")
        nodes = nodes[o]
        la = np.zeros(nb2, np.int64)
        lb = np.zeros(nb2, np.int64)
        fl = np.zeros(nb2, np.int32)
        for node in nodes:
            da, db = deg_a[node], deg_b[node]
            cand = np.maximum(la + da, lb + db).astype(np.float64)
            cand[fl >= BLK] = np.inf
            b = int(np.argmin(cand))
            perm2[half * HALF + b * BLK + fl[b]] = node
            fl[b] += 1
            la[b] += da
            lb[b] += db
    return perm2


def _host_prep(x, edge_index, W1, att_src1, att_dst1, b1, W2, att_src2, att_dst2, b2,
               n_nodes=N_NODES, n_cores=N_CORES):
    """Sort/pad edges, build per-core input maps and compile-time params."""
    NP = n_nodes // n_cores
    NB = NP // BLK
    assert NB * BLK == NP and NB % G == 0
    NSB = NB // G
    HALF = n_nodes // 2
    assert HALF < 32768 and NP < 32768

    src0 = np.concatenate([np.asarray(edge_index[0]), np.arange(n_nodes)]).astype(np.int64)
    dst0 = np.concatenate([np.asarray(edge_index[1]), np.arange(n_nodes)]).astype(np.int64)

    nblk_tot = n_cores * NB
    perm = _balance_blocks2(src0, dst0, n_nodes, nblk_tot)  # new pos -> old node
    old2new = np.empty(n_nodes, np.int64)
    old2new[perm] = np.arange(n_nodes)

    src = old2new[src0]
    dst = old2new[dst0]
    order = np.argsort(dst, kind="stable")
    src, dst = src[order], dst[order]

    blk_of = dst // BLK
    # within each dst-block, put src<HALF ("a") edges first
    order2 = np.lexsort((src >= HALF, blk_of))
    src, dst = src[order2], dst[order2]
    is_b = src >= HALF
    cnt_a = np.bincount(blk_of[order2], weights=~is_b, minlength=nblk_tot).astype(np.int64)
    cnt_b = np.bincount(blk_of[order2], weights=is_b, minlength=nblk_tot).astype(np.int64)
    starts = np.concatenate([[0], np.cumsum(cnt_a + cnt_b)]).astype(np.int64)
    Ka = int(np.ceil(cnt_a.max() / 128))
    Kb = int(np.ceil(cnt_b.max() / 128))
    K = Ka + Kb

    # per-block padded arrays in [a-pad | b-pad] chunk order
    srcA = np.zeros((nblk_tot, Ka * 128), np.int64)      # pad -> row 0
    srcB = np.zeros((nblk_tot, Kb * 128), np.int64)
    dstL = np.zeros((nblk_tot, K * 128), np.int64)       # dst local to core
    dloc = np.full((nblk_tot, K * 128), 999.0, np.float32)  # dst local to block
    for b in range(nblk_tot):
        na, nb_ = int(cnt_a[b]), int(cnt_b[b])
        s = starts[b]
        core = b // NB
        srcA[b, :na] = src[s:s + na]
        srcB[b, :nb_] = src[s + na:s + na + nb_] - HALF
        dstL[b, :na] = dst[s:s + na] - core * NP
        dstL[b, Ka * 128:Ka * 128 + nb_] = dst[s + na:s + na + nb_] - core * NP
        dloc[b, :na] = dst[s:s + na] - b * BLK
        dloc[b, Ka * 128:Ka * 128 + nb_] = dst[s + na:s + na + nb_] - b * BLK

    # shared (replicated) weights
    AA1 = np.zeros((128, 16), np.float32)
    asrc1 = np.asarray(att_src1, np.float32)
    adst1 = np.asarray(att_dst1, np.float32)
    for h in range(HEADS):
        AA1[16 * h:16 * (h + 1), h] = asrc1[h]
        AA1[16 * h:16 * (h + 1), 8 + h] = adst1[h]
    AA2 = np.stack([np.asarray(att_src2, np.float32)[0],
                    np.asarray(att_dst2, np.float32)[0]], axis=1)  # [64, 2]
    shared = {
        "W1": np.asarray(W1, np.float32),
        "AA1": AA1,
        "B1": np.tile(np.asarray(b1, np.float32), (128, 1)),
        "W2": np.asarray(W2, np.float32),
        "AA2": AA2,
        "B2": np.tile(np.asarray(b2, np.float32), (128, 1)),
        "IOTA": np.tile(np.arange(128), (128, 1)).astype(TNP),
    }

    xT = np.ascontiguousarray(np.asarray(x, np.float32).T)  # [128, N] (old order)

    in_maps = []
    for c in range(n_cores):
        lo = c * NB
        # super-block idx layout: [A(b0..) | B(b0..) | dst(b0..)] wrapped
        idx = np.stack([
            np.concatenate(
                [_wrap16(srcA[lo + s * G + g]) for g in range(G)] +
                [_wrap16(srcB[lo + s * G + g]) for g in range(G)] +
                [_wrap16(dstL[lo + s * G + g]) for g in range(G)], axis=1)
            for s in range(NSB)])
        dl = np.stack([
            np.concatenate(
                [dloc[lo + s * G + g].reshape(K, 128).T for g in range(G)], axis=1)
            for s in range(NSB)])                        # [NSB, 128, G*K]
        m = dict(shared)
        m["xT"] = np.ascontiguousarray(xT[:, perm[c * NP:(c + 1) * NP]])
        m["IDX"] = np.ascontiguousarray(idx)
        m["DLOC"] = np.ascontiguousarray(dl.astype(TNP))
        in_maps.append(m)

    prm = dict(NP=NP, NB=NB, NSB=NSB, K=K, Ka=Ka, Kb=Kb,
               n_nodes=n_nodes, n_cores=n_cores, HALF=HALF, perm=perm)
    return in_maps, prm


def _build_program(prm, repeat=1):
    import os
    SKIP_AG = bool(int(os.environ.get("BG_SKIP_AG", "0")))
    SKIP_EDGE = bool(int(os.environ.get("BG_SKIP_EDGE", "0")))
    SKIP_NODE = bool(int(os.environ.get("BG_SKIP_NODE", "0")))
    NP, NSB, K, Ka, Kb = prm["NP"], prm["NSB"], prm["K"], prm["Ka"], prm["Kb"]
    HALF = prm["HALF"]
    n_nodes, n_cores = prm["n_nodes"], prm["n_cores"]
    RG = [list(range(n_cores))]
    CW = G * (Ka + Kb + K) * 8  # idx tensor cols per super-block

    nc = bacc.Bacc("TRN2", target_bir_lowering=False, debug=False,
                   num_devices=n_cores, num_swdge_queues=4)
    qn = [0]  # round-robin SWDGE queue assignment for gathers

    def next_q():
        qn[0] += 1
        return qn[0] % 4

    # inputs
    xT = nc.dram_tensor("xT", [128, NP], F32, kind="ExternalInput")
    W1 = nc.dram_tensor("W1", [128, 128], F32, kind="ExternalInput")
    AA1 = nc.dram_tensor("AA1", [128, 16], F32, kind="ExternalInput")
    B1 = nc.dram_tensor("B1", [128, 128], F32, kind="ExternalInput")
    W2 = nc.dram_tensor("W2", [128, 64], F32, kind="ExternalInput")
    AA2 = nc.dram_tensor("AA2", [64, 2], F32, kind="ExternalInput")
    B2 = nc.dram_tensor("B2", [128, 64], F32, kind="ExternalInput")
    IOTA = nc.dram_tensor("IOTA", [128, 128], TD, kind="ExternalInput")
    IDX = nc.dram_tensor("IDX", [NSB, 128, CW], I16, kind="ExternalInput")
    DLOC = nc.dram_tensor("DLOC", [NSB, 128, G * K], TD, kind="ExternalInput")
    OUT = nc.dram_tensor("out", [NP, OUT_C], F32, kind="ExternalOutput")
    # internal DRAM
    P1L = nc.dram_tensor("P1L", [NP, P1W], TD)
    P1F = nc.dram_tensor("P1F", [n_nodes, P1W], TD, addr_space="Shared")
    P2L = nc.dram_tensor("P2L", [NP, P2W], TD)
    P2F = nc.dram_tensor("P2F", [n_nodes, P2W], TD, addr_space="Shared")

    mm = mybir.AluOpType
    ACT = mybir.ActivationFunctionType

    def gcalls(ix, out3, table, col0, nchunk, elem, estep=None):
        step = MAXI // 128
        for c0 in range(0, nchunk, step):
            c1 = min(c0 + step, nchunk)
            nc.gpsimd.dma_gather(
                out_ap=out3[:, c0:c1, :], in_ap=table,
                idxs_ap=ix[:, (col0 + c0) * 8:(col0 + c1) * 8],
                num_idxs=(c1 - c0) * 128,
                num_idxs_reg=(c1 - c0) * 128, elem_size=elem,
                elem_step=estep, queue_num=next_q())

    from contextlib import ExitStack
    with tile.TileContext(nc) as tc, ExitStack() as ctx:
        cst = ctx.enter_context(tc.tile_pool(name="cst", bufs=1))
        W1t = cst.tile([128, 128], F32)
        AA1t = cst.tile([128, 16], F32)
        B1t = cst.tile([128, 128], F32)
        W2t = cst.tile([128, 64], F32)
        AA2t = cst.tile([64, 2], F32)
        B2t = cst.tile([128, 64], F32)
        IOTAt = cst.tile([128, 128], TD)
        IDENT = cst.tile([128, 128], F32)
        for t, d in ((W1t, W1), (AA1t, AA1), (B1t, B1), (W2t, W2),
                     (AA2t, AA2), (B2t, B2), (IOTAt, IOTA)):
            nc.sync.dma_start(out=t[:], in_=d[:, :])
        make_identity(nc, IDENT[:])

        # body may be repeated for differential benchmarking
        for _rep in range(repeat):
            # ---------------- stage A: L1 node stage (512-wide tiles) ----------------
            with nc.named_scope("nodeA"), \
                 tc.tile_pool(name="pa", bufs=2) as pa, \
                 tc.tile_pool(name="ppa", bufs=2, space="PSUM") as ppa:
                for c0 in ([] if SKIP_NODE else range(0, NP, 512)):
                    nn = min(512, NP - c0)
                    ng = nn // 128          # full 128-groups
                    tail = nn - ng * 128    # ragged remainder (last tile)
                    xt = pa.tile([128, 512], F32, tag="xt")
                    nc.sync.dma_start(out=xt[:, :nn], in_=xT[:, c0:c0 + nn])
                    pm = ppa.tile([128, 512], F32, tag="pp")
                    nc.tensor.matmul(pm[:, :nn], lhsT=W1t[:], rhs=xt[:, :nn],
                                     start=True, stop=True)
                    xhT = pa.tile([128, 512], F32, tag="xhT")
                    nc.vector.tensor_copy(out=xhT[:, :nn], in_=pm[:, :nn])
                    pm2 = ppa.tile([16, 512], F32, tag="pp2")
                    nc.tensor.matmul(pm2[:, :nn], lhsT=AA1t[:], rhs=xhT[:, :nn],
                                     start=True, stop=True)
                    aaT = pa.tile([16, 512], F32, tag="aaT")
                    nc.vector.tensor_copy(out=aaT[:, :nn], in_=pm2[:, :nn])
                    ptx = ppa.tile([128, 512], F32, tag="ppx")
                    pt2 = ppa.tile([128, 64], F32, tag="pp3")
                    grps = ng + (1 if tail else 0)
                    for g in range(grps):
                        s = g * 128
                        w_ = min(128, nn - s)
                        nc.tensor.transpose(ptx[:w_, s:s + 128],
                                            xhT[:, s:s + w_], IDENT[:])
                        nc.tensor.transpose(pt2[:w_, g * 16:(g + 1) * 16],
                                            aaT[:, s:s + w_], IDENT[:16, :16])
                    xh = pa.tile([128, 512], TD, tag="xh")
                    nc.vector.tensor_copy(out=xh[:], in_=ptx[:])
                    aa = pa.tile([128, 512], TD, tag="aa")
                    nc.vector.memset(aa[:], 0.0)
                    nc.vector.tensor_copy(
                        out=aa[:].rearrange("n (g w) -> n g w",
                                            w=128)[:, :grps, 0:16],
                        in_=pt2[:, 0:grps * 16].rearrange(
                            "n (g w) -> n g w", w=16))
                    for g in range(grps):
                        s = g * 128
                        w_ = min(128, nn - s)
                        nc.sync.dma_start(out=P1L[c0 + s:c0 + s + w_, 0:128],
                                          in_=xh[:w_, s:s + 128])
                        nc.sync.dma_start(out=P1L[c0 + s:c0 + s + w_, 128:P1W],
                                          in_=aa[:w_, s:s + 128])

            with nc.named_scope("ag1"):
                if not SKIP_AG:
                    nc.gpsimd.collective_compute(
                        "AllGather", mm.bypass, replica_groups=RG,
                        ins=[P1L[:, :]], outs=[P1F[:, :]])

            # ---------------- L1 edge stage (+ fused L2 node stage) ----------------
            with nc.named_scope("edge1"), \
                 tc.tile_pool(name="gma", bufs=2) as gmap, \
                 tc.tile_pool(name="gmb", bufs=2) as gmbp, \
                 tc.tile_pool(name="gad", bufs=2) as gadp, \
                 tc.tile_pool(name="off", bufs=2) as offp, \
                 tc.tile_pool(name="sml", bufs=3) as sml, \
                 tc.tile_pool(name="sal", bufs=2) as salp, \
                 tc.tile_pool(name="hb", bufs=1) as hbp, \
                 tc.tile_pool(name="a2", bufs=1) as a2p, \
                 tc.tile_pool(name="ps1", bufs=2, space="PSUM") as ps1p, \
                 tc.tile_pool(name="psa2", bufs=1, space="PSUM") as psa2p:

                for sb in ([] if SKIP_EDGE else range(NSB)):
                    ix = offp.tile([128, CW], I16, tag="ix")
                    nc.sync.dma_start(out=ix[:], in_=IDX[sb, :, :])
                    gmA = gmap.tile([128, G * Ka * P1W], TD, tag="gmA")
                    gmA3 = gmA[:].rearrange("p (r w) -> p r w", w=P1W)
                    gmA4 = gmA[:].rearrange("p (g k w) -> p g k w", g=G, w=P1W)
                    gcalls(ix, gmA3, P1F[0:HALF, :], 0, G * Ka, P1W)
                    gmB = gmbp.tile([128, G * Kb * P1W], TD, tag="gmB")
                    gmB3 = gmB[:].rearrange("p (r w) -> p r w", w=P1W)
                    gmB4 = gmB[:].rearrange("p (g k w) -> p g k w", g=G, w=P1W)
                    gcalls(ix, gmB3, P1F[HALF:n_nodes, :], G * Ka, G * Kb, P1W)
                    gad = gadp.tile([128, G * K * (P1W - 128)], TD, tag="gad")
                    gad3 = gad[:].rearrange("p (r w) -> p r w", w=P1W - 128)
                    gad4 = gad[:].rearrange("p (g k w) -> p g k w", g=G,
                                            w=P1W - 128)
                    gcalls(ix, gad3, P1L[:, 128:P1W], G * (Ka + Kb), G * K,
                           P1W - 128, estep=P1W)
                    dl = sml.tile([128, G * K], TD, tag="dl")
                    nc.sync.dma_start(out=dl[:], in_=DLOC[sb, :, :])

                    # whole-super-block score pipeline (a few fat instructions)
                    eaA = sml.tile([128, G * Ka * 8], F32, tag="eaA")
                    eaA4 = eaA[:].rearrange("p (g k h) -> p g k h", g=G, h=8)
                    nc.vector.tensor_tensor(
                        out=eaA4, in0=gmA4[:, :, :, 128:136],
                        in1=gad4[:, :, 0:Ka, 8:16], op=mm.add)
                    eaB = sml.tile([128, G * Kb * 8], F32, tag="eaB")
                    eaB4 = eaB[:].rearrange("p (g k h) -> p g k h", g=G, h=8)
                    nc.vector.tensor_tensor(
                        out=eaB4, in0=gmB4[:, :, :, 128:136],
                        in1=gad4[:, :, Ka:K, 8:16], op=mm.add)
                    tlA = sml.tile([128, G * Ka * 8], F32, tag="tlA")
                    nc.vector.tensor_scalar_mul(tlA[:], eaA[:], NEG_SLOPE)
                    nc.vector.tensor_tensor(out=eaA[:], in0=eaA[:], in1=tlA[:],
                                            op=mm.max)
                    tlB = sml.tile([128, G * Kb * 8], F32, tag="tlB")
                    nc.vector.tensor_scalar_mul(tlB[:], eaB[:], NEG_SLOPE)
                    nc.vector.tensor_tensor(out=eaB[:], in0=eaB[:], in1=tlB[:],
                                            op=mm.max)
                    nc.scalar.activation(out=gmA4[:, :, :, 128:136], in_=eaA4,
                                         func=ACT.Exp)
                    nc.scalar.activation(out=gmB4[:, :, :, 128:136], in_=eaB4,
                                         func=ACT.Exp)
                    # msg = xh * ex, whole super-block per half
                    nc.vector.tensor_tensor(
                        out=gmA3[:, :, 0:128].rearrange(
                            "p r (h c) -> p r h c", c=16),
                        in0=gmA3[:, :, 0:128].rearrange(
                            "p r (h c) -> p r h c", c=16),
                        in1=gmA3[:, :, 128:136].rearrange(
                            "p r (h o) -> p r h o", o=1).to_broadcast(
                            [128, G * Ka, 8, 16]),
                        op=mm.mult)
                    nc.vector.tensor_tensor(
                        out=gmB3[:, :, 0:128].rearrange(
                            "p r (h c) -> p r h c", c=16),
                        in0=gmB3[:, :, 0:128].rearrange(
                            "p r (h c) -> p r h c", c=16),
                        in1=gmB3[:, :, 128:136].rearrange(
                            "p r (h o) -> p r h o", o=1).to_broadcast(
                            [128, G * Kb, 8, 16]),
                        op=mm.mult)

                    hball = hbp.tile([128, G * 136], F32, tag="hball")
                    hb4 = hball[:].rearrange("p (g w) -> p g w", g=G)
                    for bi in range(G):
                        S_all = salp.tile([128, K * 128], TD, tag="sall")
                        nc.vector.tensor_tensor(
                            out=S_all[:].rearrange("p (k e) -> p k e", e=128),
                            in0=IOTAt[:].rearrange(
                                "p (o e) -> p o e", o=1).to_broadcast(
                                [128, K, 128]),
                            in1=dl[:, bi * K:(bi + 1) * K].rearrange(
                                "p (k o) -> p k o", o=1).to_broadcast(
                                [128, K, 128]),
                            op=mm.is_equal)
                        ps = ps1p.tile([128, 136], F32, tag="ps")
                        for k in range(K):
                            if k < Ka:
                                r = bi * Ka + k
                                ck = gmA[:, r * P1W:r * P1W + 136]
                            else:
                                r = bi * Kb + (k - Ka)
                                ck = gmB[:, r * P1W:r * P1W + 136]
                            nc.tensor.matmul(
                                ps[:], lhsT=S_all[:, k * 128:(k + 1) * 128],
                                rhs=ck, start=(k == 0), stop=(k == K - 1))
                        nc.vector.tensor_copy(
                            out=hball[:, bi * 136:(bi + 1) * 136], in_=ps[:])
                    # batched epilogue over the super-block:
                    # h = msg/den + b1 ; elu  (in place over hb4[:, :, 0:128])
                    rd = sml.tile([128, G * 8], F32, tag="rd")
                    rd3 = rd[:].rearrange("p (g h) -> p g h", g=G)
                    nc.vector.tensor_scalar_add(rd3, hb4[:, :, 128:136], EPS)
                    nc.vector.reciprocal(rd[:], rd[:])
                    hm = hbp.tile([128, G * 128], F32, tag="hm")
                    hm4 = hm[:].rearrange("p (g h c) -> p g h c", g=G, c=16)
                    nc.vector.tensor_tensor(
                        out=hm4,
                        in0=hb4[:, :, 0:128].rearrange(
                            "p g (h c) -> p g h c", c=16),
                        in1=rd3.rearrange("p g (h o) -> p g h o",
                                          o=1).to_broadcast([128, G, 8, 16]),
                        op=mm.mult)
                    nc.vector.tensor_tensor(
                        out=hm[:].rearrange("p (g w) -> p g w", g=G),
                        in0=hm[:].rearrange("p (g w) -> p g w", g=G),
                        in1=B1t[:].rearrange("p (o w) -> p o w",
                                             o=1).to_broadcast([128, G, 128]),
                        op=mm.add)
                    tm = hbp.tile([128, G * 128], F32, tag="tm")
                    nc.vector.tensor_scalar_min(tm[:], hm[:], 0.0)
                    nc.scalar.activation(out=tm[:], in_=tm[:], func=ACT.Exp)
                    nc.vector.tensor_scalar_sub(tm[:], tm[:], 1.0)
                    nc.vector.tensor_tensor(out=hm[:], in0=hm[:], in1=tm[:],
                                            op=mm.max)
                    # batched L2 node stage: hT (packed valid slots), then
                    # x2 = W2^T @ hT and a2 = AA2^T @ x2T, written to P2L by
                    # transposing DMAs
                    hTall = a2p.tile([128, G * BLK], F32, tag="hTall")
                    for bi in range(G):
                        pt = psa2p.tile([128, 128], F32, tag="pa2")
                        nc.tensor.transpose(pt[:, :BLK],
                                            hm[:BLK, bi * 128:(bi + 1) * 128],
                                            IDENT[:BLK, :BLK])
                        nc.vector.tensor_copy(
                            out=hTall[:, bi * BLK:(bi + 1) * BLK],
                            in_=pt[:, :BLK])
                    x2T = a2p.tile([64, G * BLK], F32, tag="x2T")
                    for c0 in range(0, G * BLK, 512):
                        c1 = min(c0 + 512, G * BLK)
                        p2m = psa2p.tile([64, 512], F32, tag="p2m")
                        nc.tensor.matmul(p2m[:, :c1 - c0], lhsT=W2t[:],
                                         rhs=hTall[:, c0:c1],
                                         start=True, stop=True)
                        nc.vector.tensor_copy(out=x2T[:, c0:c1],
                                              in_=p2m[:, :c1 - c0])
                    a2T = a2p.tile([2, G * BLK], F32, tag="a2T")
                    for c0 in range(0, G * BLK, 512):
                        c1 = min(c0 + 512, G * BLK)
                        p2a = psa2p.tile([2, 512], F32, tag="p2a")
                        nc.tensor.matmul(p2a[:, :c1 - c0], lhsT=AA2t[:],
                                         rhs=x2T[:, c0:c1],
                                         start=True, stop=True)
                        nc.vector.tensor_copy(out=a2T[:, c0:c1],
                                              in_=p2a[:, :c1 - c0])
                    x2r = a2p.tile([128, G * 64], TD, tag="x2r")
                    a2r = a2p.tile([128, G * 2], TD, tag="a2r")
                    for bi in range(G):
                        p2t = psa2p.tile([128, 64], F32, tag="p2t")
                        nc.tensor.transpose(p2t[:BLK, :],
                                            x2T[:, bi * BLK:(bi + 1) * BLK],
                                            IDENT[:64, :64])
                        nc.vector.tensor_copy(
                            out=x2r[:BLK, bi * 64:(bi + 1) * 64],
                            in_=p2t[:BLK, :])
                        p2u = psa2p.tile([128, 2], F32, tag="p2u")
                        nc.tensor.transpose(p2u[:BLK, :],
                                            a2T[:, bi * BLK:(bi + 1) * BLK],
                                            IDENT[:2, :2])
                        nc.vector.tensor_copy(
                            out=a2r[:BLK, bi * 2:(bi + 1) * 2],
                            in_=p2u[:BLK, :])
                    n0 = sb * G * BLK
                    nc.sync.dma_start(
                        out=P2L[n0:n0 + G * BLK, 0:64].rearrange(
                            "(g n) w -> n g w", n=BLK),
                        in_=x2r[:BLK, :].rearrange("n (g w) -> n g w", g=G))
                    nc.sync.dma_start(
                        out=P2L[n0:n0 + G * BLK, 64:66].rearrange(
                            "(g n) w -> n g w", n=BLK),
                        in_=a2r[:BLK, :].rearrange("n (g w) -> n g w", g=G))

            with nc.named_scope("ag2"):
                if not SKIP_AG:
                    nc.gpsimd.collective_compute(
                        "AllGather", mm.bypass, replica_groups=RG,
                        ins=[P2L[:, :]], outs=[P2F[:, :]])

            # ---------------- L2 edge stage ----------------
            with nc.named_scope("edge2"), \
                 tc.tile_pool(name="gma2", bufs=2) as gmap2, \
                 tc.tile_pool(name="gmb2", bufs=2) as gmbp2, \
                 tc.tile_pool(name="gad2", bufs=2) as gadp2, \
                 tc.tile_pool(name="off2", bufs=2) as offp2, \
                 tc.tile_pool(name="sml2", bufs=3) as sml2, \
                 tc.tile_pool(name="sal2", bufs=2) as salp2, \
                 tc.tile_pool(name="ob", bufs=2) as obp, \
                 tc.tile_pool(name="ps2", bufs=2, space="PSUM") as ps2p:

                for sb in ([] if SKIP_EDGE else range(NSB)):
                    ix = offp2.tile([128, CW], I16, tag="ix2")
                    nc.sync.dma_start(out=ix[:], in_=IDX[sb, :, :])
                    gmA = gmap2.tile([128, G * Ka * P2W], TD, tag="gmA2")
                    gmA3 = gmA[:].rearrange("p (r w) -> p r w", w=P2W)
                    gmA4 = gmA[:].rearrange("p (g k w) -> p g k w", g=G, w=P2W)
                    gcalls(ix, gmA3, P2F[0:HALF, :], 0, G * Ka, P2W)
                    gmB = gmbp2.tile([128, G * Kb * P2W], TD, tag="gmB2")
                    gmB3 = gmB[:].rearrange("p (r w) -> p r w", w=P2W)
                    gmB4 = gmB[:].rearrange("p (g k w) -> p g k w", g=G, w=P2W)
                    gcalls(ix, gmB3, P2F[HALF:n_nodes, :], G * Ka, G * Kb, P2W)
                    gad = gadp2.tile([128, G * K * P2W], TD, tag="gad2")
                    gad3 = gad[:].rearrange("p (r w) -> p r w", w=P2W)
                    gad4 = gad[:].rearrange("p (g k w) -> p g k w", g=G, w=P2W)
                    gcalls(ix, gad3, P2L[:, :], G * (Ka + Kb), G * K, P2W)
                    dl = sml2.tile([128, G * K], TD, tag="dl2")
                    nc.sync.dma_start(out=dl[:], in_=DLOC[sb, :, :])

                    eaA = sml2.tile([128, G * Ka], F32, tag="eaA2")
                    eaA3 = eaA[:].rearrange("p (g k) -> p g k", g=G)
                    nc.vector.tensor_tensor(
                        out=eaA3,
                        in0=gmA4[:, :, :, 64:65].rearrange("p g k o -> p g (k o)"),
                        in1=gad4[:, :, 0:Ka, 65:66].rearrange("p g k o -> p g (k o)"),
                        op=mm.add)
                    eaB = sml2.tile([128, G * Kb], F32, tag="eaB2")
                    eaB3 = eaB[:].rearrange("p (g k) -> p g k", g=G)
                    nc.vector.tensor_tensor(
                        out=eaB3,
                        in0=gmB4[:, :, :, 64:65].rearrange("p g k o -> p g (k o)"),
                        in1=gad4[:, :, Ka:K, 65:66].rearrange("p g k o -> p g (k o)"),
                        op=mm.add)
                    tlA = sml2.tile([128, G * Ka], F32, tag="tlA2")
                    nc.vector.tensor_scalar_mul(tlA[:], eaA[:], NEG_SLOPE)
                    nc.vector.tensor_tensor(out=eaA[:], in0=eaA[:], in1=tlA[:],
                                            op=mm.max)
                    tlB = sml2.tile([128, G * Kb], F32, tag="tlB2")
                    nc.vector.tensor_scalar_mul(tlB[:], eaB[:], NEG_SLOPE)
                    nc.vector.tensor_tensor(out=eaB[:], in0=eaB[:], in1=tlB[:],
                                            op=mm.max)
                    nc.scalar.activation(
                        out=gmA4[:, :, :, 64:65].rearrange("p g k o -> p g (k o)"),
                        in_=eaA3, func=ACT.Exp)
                    nc.scalar.activation(
                        out=gmB4[:, :, :, 64:65].rearrange("p g k o -> p g (k o)"),
                        in_=eaB3, func=ACT.Exp)
                    nc.vector.tensor_tensor(
                        out=gmA3[:, :, 0:64], in0=gmA3[:, :, 0:64],
                        in1=gmA3[:, :, 64:65].to_broadcast([128, G * Ka, 64]),
                        op=mm.mult)
                    nc.vector.tensor_tensor(
                        out=gmB3[:, :, 0:64], in0=gmB3[:, :, 0:64],
                        in1=gmB3[:, :, 64:65].to_broadcast([128, G * Kb, 64]),
                        op=mm.mult)

                    oball = obp.tile([128, G * 65], F32, tag="oball")
                    ob4 = oball[:].rearrange("p (g w) -> p g w", g=G)
                    for bi in range(G):
                        S_all = salp2.tile([128, K * 128], TD, tag="sall2")
                        nc.vector.tensor_tensor(
                            out=S_all[:].rearrange("p (k e) -> p k e", e=128),
                            in0=IOTAt[:].rearrange(
                                "p (o e) -> p o e", o=1).to_broadcast(
                                [128, K, 128]),
                            in1=dl[:, bi * K:(bi + 1) * K].rearrange(
                                "p (k o) -> p k o", o=1).to_broadcast(
                                [128, K, 128]),
                            op=mm.is_equal)
                        ps = ps2p.tile([128, 65], F32, tag="psb")
                        for k in range(K):
                            if k < Ka:
                                r = bi * Ka + k
                                ck = gmA[:, r * P2W:r * P2W + 65]
                            else:
                                r = bi * Kb + (k - Ka)
                                ck = gmB[:, r * P2W:r * P2W + 65]
                            nc.tensor.matmul(
                                ps[:], lhsT=S_all[:, k * 128:(k + 1) * 128],
                                rhs=ck, start=(k == 0), stop=(k == K - 1))
                        nc.vector.tensor_copy(
                            out=oball[:, bi * 65:(bi + 1) * 65], in_=ps[:])
                    rd = sml2.tile([128, G], F32, tag="rd2")
                    rd3 = rd[:].rearrange("p (g o) -> p g o", o=1)
                    nc.vector.tensor_scalar_add(rd3, ob4[:, :, 64:65], EPS)
                    nc.vector.reciprocal(rd[:], rd[:])
                    ob = obp.tile([128, G * 64], F32, tag="ob")
                    ob3 = ob[:].rearrange("p (g w) -> p g w", g=G)
                    nc.vector.tensor_tensor(
                        out=ob3, in0=ob4[:, :, 0:64],
                        in1=rd3.to_broadcast([128, G, 64]), op=mm.mult)
                    nc.vector.tensor_tensor(
                        out=ob3, in0=ob3,
                        in1=B2t[:].rearrange("p (o w) -> p o w",
                                             o=1).to_broadcast([128, G, 64]),
                        op=mm.add)
                    n0 = sb * G * BLK
                    nc.sync.dma_start(
                        out=OUT[n0:n0 + G * BLK, :].rearrange(
                            "(g n) w -> n g w", n=BLK),
                        in_=ob[:BLK, :].rearrange("n (g w) -> n g w", g=G))

    nc.compile()
    return nc


def _run(inputs, sim=False, trace=False):
    in_maps, prm = _host_prep(**inputs)
    nc = _build_program(prm)
    n_cores = prm["n_cores"]
    if sim:
        from concourse.bass_interp import MultiCoreSim
        ms = MultiCoreSim(nc, num_cores=n_cores)
        for c in range(n_cores):
            for k, v in in_maps[c].items():
                ms.cores[c].tensor(k)[:] = v
        ms.simulate()
        outs = [np.array(ms.cores[c].tensor("out")) for c in range(n_cores)]
        got = np.concatenate(outs, axis=0)
        res = None
    else:
        from concourse.bass_utils import run_bass_kernel_spmd
        res = run_bass_kernel_spmd(nc, in_maps, core_ids=list(range(n_cores)),
                                   trace=trace)
        outs = [res.results[c]["out"] for c in range(n_cores)]
        got = np.concatenate(outs, axis=0)
    full = np.empty_like(got)
    full[prm["perm"]] = got
    return full, res


def kernel(**inputs):
    out, _ = _run({k: np.asarray(v) for k, v in inputs.items()})
    return out


# revision 39
# speedup vs baseline: 3.7393x; 1.1499x over previous
"""BiGAT (2-layer GAT, PyG-style with self-loops) on 8 Trainium2 NeuronCores.

Strategy: partition nodes (and their incoming edges) by destination across 8
cores. Nodes are permuted so every 125-node dst block carries a near-equal
edge count in both src-halves (two-pass balancing), making the per-block
chunk count uniform and small. Edges are sorted by dst and padded to a
uniform blocks-x-chunks structure so one SPMD program serves all cores.

The measured runtime cost structure on this stack is ~100us PER INSTRUCTION
(dispatch-bound; payload size nearly free), so the kernel minimizes the
instruction count:
  - gathers batched into maximal 1024-idx calls spanning G=5 blocks;
  - per-edge score/softmax vector work merged into a handful of whole-
    super-block instructions (strided multi-dim access patterns);
  - the one-hot scatter matrices for a block are built by a single
    is_equal; gathered rows are packed [xh | a_src | a_dst | pad] so the
    exp() output can overwrite a_src in place and each chunk's scatter is
    ONE PSUM-accumulating matmul over [msg | ex].

Per layer:
  node stage : xh = x @ W and attention dots (PE); packed rows written to a
               local DRAM table; AllGather replicates the table.
  edge stage : dma_gather rows by src and [a|..] rows by dst-local;
               e = lrelu(a_src+a_dst); ex = exp(e) (softmax max-shift
               skipped -- scores are O(10) so exp cannot overflow, and
               softmax is shift-invariant); msg = xh_src * ex; one-hot
               matmul scatter-adds [msg | ex] into PSUM per block; epilogue
               divides by the summed ex, adds bias. The L2 node stage is
               fused into the L1 edge epilogue.

dma_gather constraints honored: int16 indices (src tables split into two
<=25000-row halves; dst uses core-local indices), <=1024 idxs per call,
row strides and elem sizes multiples of 256B, indices wrapped [16, n/16]
and replicated to 128 partitions.
"""
import sys

sys.path.insert(0, "/opt/trn_rl_repo")

import heapq
import numpy as np

from concourse import bass, mybir
import concourse.bacc as bacc
import concourse.tile as tile
from concourse.masks import make_identity

F32 = mybir.dt.float32
I16 = mybir.dt.int16
TD = mybir.dt.bfloat16
import ml_dtypes
TNP = ml_dtypes.bfloat16

# ---------------- problem constants (hardcoded per contract) ----------------
N_NODES = 50000
N_EDGES = 800000
IN_C, HID_C, OUT_C, HEADS = 128, 16, 64, 8
NEG_SLOPE = 0.2
N_CORES = 8

# ---------------- sharding / tiling parameters ----------------
BLK = 125       # dst nodes per edge-stage block (<=128 for one-hot)
P1W = 256       # L1 table row: [xh(128) | a_src(8) | a_dst(8) | pad]
P2W = 128       # L2 table row: [x2(64) | a_src2(1) | a_dst2(1) | pad]
G = 5           # blocks per super-block (gather batching unit)
MAXI = 1024     # max idxs per dma_gather call (HW cap; >1024 crashes)
EPS = 1e-16


def _wrap16(idx):
    """[L] int array -> dma_gather wrapped layout [128, L//16] int16."""
    L = len(idx)
    w = idx.reshape(L // 16, 16).T
    return np.tile(w, (8, 1)).astype(np.int16)


def _balance_blocks2(src0, dst0, n_nodes, nblk_tot):
    """Two-pass node->block assignment: pass 1 equalizes total in-degree
    (fixing each node's half); pass 2 reassigns within each half to equalize
    per-block edge counts from BOTH halves. Returns perm (new pos -> node)."""
    HALF = n_nodes // 2
    deg = np.bincount(dst0, minlength=n_nodes)
    order = np.argsort(-deg, kind="stable")
    fill = np.zeros(nblk_tot, np.int32)
    perm = np.empty(n_nodes, np.int64)
    heap = [(0, b) for b in range(nblk_tot)]
    heapq.heapify(heap)
    for node in order:
        load, b = heapq.heappop(heap)
        perm[b * BLK + fill[b]] = node
        fill[b] += 1
        load += int(deg[node])
        if fill[b] < BLK:
            heapq.heappush(heap, (load, b))

    old2new = np.empty(n_nodes, np.int64)
    old2new[perm] = np.arange(n_nodes)
    src_in_a = old2new[src0] < HALF
    deg_a = np.bincount(dst0, weights=src_in_a.astype(np.float64),
                        minlength=n_nodes).astype(np.int64)
    deg_b = deg - deg_a

    perm2 = np.empty(n_nodes, np.int64)
    nb2 = nblk_tot // 2
    for half in (0, 1):
        nodes = perm[half * HALF:(half + 1) * HALF]
        o = np.argsort(-(deg_a[nodes] + deg_b[nodes]), kind="stable")
        nodes = nodes[o]
        la = np.zeros(nb2, np.int64)
        lb = np.zeros(nb2, np.int64)
        fl = np.zeros(nb2, np.int32)
        for node in nodes:
            da, db = deg_a[node], deg_b[node]
            cand = np.maximum(la + da, lb + db).astype(np.float64)
            cand[fl >= BLK] = np.inf
            b = int(np.argmin(cand))
            perm2[half * HALF + b * BLK + fl[b]] = node
            fl[b] += 1
            la[b] += da
            lb[b] += db
    # repair pass: swap nodes between over-cap and low-load blocks until both
    # per-block half-counts fit in 9 chunks
    CAP = 9 * 128
    la = np.zeros(nblk_tot, np.int64)
    lb = np.zeros(nblk_tot, np.int64)
    for b in range(nblk_tot):
        nodes = perm2[b * BLK:(b + 1) * BLK]
        la[b] = deg_a[nodes].sum()
        lb[b] = deg_b[nodes].sum()
    for half in (0, 1):
        bs = np.arange(half * nb2, (half + 1) * nb2)
        for _ in range(3000):
            w = bs[int(np.argmax(np.maximum(la[bs], lb[bs])))]
            if max(la[w], lb[w]) <= CAP:
                break
            dim_a = la[w] >= lb[w]
            key = deg_a if dim_a else deg_b
            wn = perm2[w * BLK:(w + 1) * BLK]
            iw = int(np.argmax(key[wn]))
            nw = wn[iw]
            order = np.argsort(la[bs] if dim_a else lb[bs])
            done = False
            for d in bs[order[:20]]:
                dn = perm2[d * BLK:(d + 1) * BLK]
                idn = int(np.argmin(key[dn]))
                nd = dn[idn]
                dla = deg_a[nw] - deg_a[nd]
                dlb = deg_b[nw] - deg_b[nd]
                if ((dla if dim_a else dlb) > 0 and la[d] + dla <= CAP
                        and lb[d] + dlb <= CAP):
                    perm2[w * BLK + iw], perm2[d * BLK + idn] = nd, nw
                    la[w] -= dla
                    lb[w] -= dlb
                    la[d] += dla
                    lb[d] += dlb
                    done = True
                    break
            if not done:
                break
    return perm2


def _host_prep(x, edge_index, W1, att_src1, att_dst1, b1, W2, att_src2, att_dst2, b2,
               n_nodes=N_NODES, n_cores=N_CORES):
    """Sort/pad edges, build per-core input maps and compile-time params."""
    NP = n_nodes // n_cores
    NB = NP // BLK
    assert NB * BLK == NP and NB % G == 0
    NSB = NB // G
    HALF = n_nodes // 2
    assert HALF < 32768 and NP < 32768

    src0 = np.concatenate([np.asarray(edge_index[0]), np.arange(n_nodes)]).astype(np.int64)
    dst0 = np.concatenate([np.asarray(edge_index[1]), np.arange(n_nodes)]).astype(np.int64)

    nblk_tot = n_cores * NB
    perm = _balance_blocks2(src0, dst0, n_nodes, nblk_tot)  # new pos -> old node
    old2new = np.empty(n_nodes, np.int64)
    old2new[perm] = np.arange(n_nodes)

    src = old2new[src0]
    dst = old2new[dst0]
    order = np.argsort(dst, kind="stable")
    src, dst = src[order], dst[order]

    blk_of = dst // BLK
    # within each dst-block, put src<HALF ("a") edges first
    order2 = np.lexsort((src >= HALF, blk_of))
    src, dst = src[order2], dst[order2]
    is_b = src >= HALF
    cnt_a = np.bincount(blk_of[order2], weights=~is_b, minlength=nblk_tot).astype(np.int64)
    cnt_b = np.bincount(blk_of[order2], weights=is_b, minlength=nblk_tot).astype(np.int64)
    starts = np.concatenate([[0], np.cumsum(cnt_a + cnt_b)]).astype(np.int64)
    Ka = int(np.ceil(cnt_a.max() / 128))
    Kb = int(np.ceil(cnt_b.max() / 128))
    K = Ka + Kb

    # per-block padded arrays in [a-pad | b-pad] chunk order
    srcA = np.zeros((nblk_tot, Ka * 128), np.int64)      # pad -> row 0
    srcB = np.zeros((nblk_tot, Kb * 128), np.int64)
    dstL = np.zeros((nblk_tot, K * 128), np.int64)       # dst local to core
    dloc = np.full((nblk_tot, K * 128), 999.0, np.float32)  # dst local to block
    for b in range(nblk_tot):
        na, nb_ = int(cnt_a[b]), int(cnt_b[b])
        s = starts[b]
        core = b // NB
        srcA[b, :na] = src[s:s + na]
        srcB[b, :nb_] = src[s + na:s + na + nb_] - HALF
        dstL[b, :na] = dst[s:s + na] - core * NP
        dstL[b, Ka * 128:Ka * 128 + nb_] = dst[s + na:s + na + nb_] - core * NP
        dloc[b, :na] = dst[s:s + na] - b * BLK
        dloc[b, Ka * 128:Ka * 128 + nb_] = dst[s + na:s + na + nb_] - b * BLK

    # shared (replicated) weights
    AA1 = np.zeros((128, 16), np.float32)
    asrc1 = np.asarray(att_src1, np.float32)
    adst1 = np.asarray(att_dst1, np.float32)
    for h in range(HEADS):
        AA1[16 * h:16 * (h + 1), h] = asrc1[h]
        AA1[16 * h:16 * (h + 1), 8 + h] = adst1[h]
    AA2 = np.stack([np.asarray(att_src2, np.float32)[0],
                    np.asarray(att_dst2, np.float32)[0]], axis=1)  # [64, 2]
    shared = {
        "W1": np.asarray(W1, np.float32),
        "AA1": AA1,
        "B1": np.tile(np.asarray(b1, np.float32), (128, 1)),
        "W2": np.asarray(W2, np.float32),
        "AA2": AA2,
        "B2": np.tile(np.asarray(b2, np.float32), (128, 1)),
        "IOTA": np.tile(np.arange(128), (128, 1)).astype(TNP),
    }

    xT = np.ascontiguousarray(np.asarray(x, np.float32).T)  # [128, N] (old order)

    in_maps = []
    for c in range(n_cores):
        lo = c * NB
        # super-block idx layout: [A(b0..) | B(b0..) | dst(b0..)] wrapped
        idx = np.stack([
            np.concatenate(
                [_wrap16(srcA[lo + s * G + g]) for g in range(G)] +
                [_wrap16(srcB[lo + s * G + g]) for g in range(G)] +
                [_wrap16(dstL[lo + s * G + g]) for g in range(G)], axis=1)
            for s in range(NSB)])
        dl = np.stack([
            np.concatenate(
                [dloc[lo + s * G + g].reshape(K, 128).T for g in range(G)], axis=1)
            for s in range(NSB)])                        # [NSB, 128, G*K]
        dl_i16 = dl.astype(TNP).view(np.int16)
        m = dict(shared)
        m["xT"] = np.ascontiguousarray(xT[:, perm[c * NP:(c + 1) * NP]])
        m["IDX"] = np.ascontiguousarray(
            np.concatenate([idx, dl_i16], axis=2))
        in_maps.append(m)

    prm = dict(NP=NP, NB=NB, NSB=NSB, K=K, Ka=Ka, Kb=Kb,
               n_nodes=n_nodes, n_cores=n_cores, HALF=HALF, perm=perm)
    return in_maps, prm


def _build_program(prm, repeat=1):
    import os
    SKIP_AG = bool(int(os.environ.get("BG_SKIP_AG", "0")))
    SKIP_EDGE = bool(int(os.environ.get("BG_SKIP_EDGE", "0")))
    SKIP_NODE = bool(int(os.environ.get("BG_SKIP_NODE", "0")))
    NP, NSB, K, Ka, Kb = prm["NP"], prm["NSB"], prm["K"], prm["Ka"], prm["Kb"]
    HALF = prm["HALF"]
    n_nodes, n_cores = prm["n_nodes"], prm["n_cores"]
    RG = [list(range(n_cores))]
    CW = G * (Ka + Kb + K) * 8  # idx cols per super-block (+G*K dloc cols)

    nc = bacc.Bacc("TRN2", target_bir_lowering=False, debug=False,
                   num_devices=n_cores, num_swdge_queues=4)
    qn = [0]  # round-robin SWDGE queue assignment for gathers

    def next_q():
        qn[0] += 1
        return qn[0] % 4

    # inputs
    xT = nc.dram_tensor("xT", [128, NP], F32, kind="ExternalInput")
    W1 = nc.dram_tensor("W1", [128, 128], F32, kind="ExternalInput")
    AA1 = nc.dram_tensor("AA1", [128, 16], F32, kind="ExternalInput")
    B1 = nc.dram_tensor("B1", [128, 128], F32, kind="ExternalInput")
    W2 = nc.dram_tensor("W2", [128, 64], F32, kind="ExternalInput")
    AA2 = nc.dram_tensor("AA2", [64, 2], F32, kind="ExternalInput")
    B2 = nc.dram_tensor("B2", [128, 64], F32, kind="ExternalInput")
    IOTA = nc.dram_tensor("IOTA", [128, 128], TD, kind="ExternalInput")
    IDX = nc.dram_tensor("IDX", [NSB, 128, CW + G * K], I16,
                         kind="ExternalInput")
    OUT = nc.dram_tensor("out", [NP, OUT_C], F32, kind="ExternalOutput")
    # internal DRAM
    P1L = nc.dram_tensor("P1L", [NP, P1W], TD)
    P1F = nc.dram_tensor("P1F", [n_nodes, P1W], TD, addr_space="Shared")
    P2L = nc.dram_tensor("P2L", [NP, P2W], TD)
    P2F = nc.dram_tensor("P2F", [n_nodes, P2W], TD, addr_space="Shared")

    mm = mybir.AluOpType
    ACT = mybir.ActivationFunctionType

    def gcalls(ix, out3, table, col0, nchunk, elem, estep=None):
        step = MAXI // 128
        for c0 in range(0, nchunk, step):
            c1 = min(c0 + step, nchunk)
            nc.gpsimd.dma_gather(
                out_ap=out3[:, c0:c1, :], in_ap=table,
                idxs_ap=ix[:, (col0 + c0) * 8:(col0 + c1) * 8],
                num_idxs=(c1 - c0) * 128,
                num_idxs_reg=(c1 - c0) * 128, elem_size=elem,
                elem_step=estep, queue_num=next_q())

    from contextlib import ExitStack
    with tile.TileContext(nc) as tc, ExitStack() as ctx:
        cst = ctx.enter_context(tc.tile_pool(name="cst", bufs=1))
        W1t = cst.tile([128, 128], F32)
        AA1t = cst.tile([128, 16], F32)
        B1t = cst.tile([128, 128], F32)
        W2t = cst.tile([128, 64], F32)
        AA2t = cst.tile([64, 2], F32)
        B2t = cst.tile([128, 64], F32)
        IOTAt = cst.tile([128, 128], TD)
        IDENT = cst.tile([128, 128], F32)
        for t, d in ((W1t, W1), (AA1t, AA1), (B1t, B1), (W2t, W2),
                     (AA2t, AA2), (B2t, B2), (IOTAt, IOTA)):
            nc.sync.dma_start(out=t[:], in_=d[:, :])
        make_identity(nc, IDENT[:])

        # body may be repeated for differential benchmarking
        for _rep in range(repeat):
            # ---------------- stage A: L1 node stage (512-wide tiles) ----------------
            with nc.named_scope("nodeA"), \
                 tc.tile_pool(name="pa", bufs=2) as pa, \
                 tc.tile_pool(name="ppa", bufs=2, space="PSUM") as ppa:
                for c0 in ([] if SKIP_NODE else range(0, NP, 512)):
                    nn = min(512, NP - c0)
                    ng = nn // 128          # full 128-groups
                    tail = nn - ng * 128    # ragged remainder (last tile)
                    xt = pa.tile([128, 512], F32, tag="xt")
                    nc.sync.dma_start(out=xt[:, :nn], in_=xT[:, c0:c0 + nn])
                    pm = ppa.tile([128, 512], F32, tag="pp")
                    nc.tensor.matmul(pm[:, :nn], lhsT=W1t[:], rhs=xt[:, :nn],
                                     start=True, stop=True)
                    xhT = pa.tile([128, 512], F32, tag="xhT")
                    nc.vector.tensor_copy(out=xhT[:, :nn], in_=pm[:, :nn])
                    pm2 = ppa.tile([16, 512], F32, tag="pp2")
                    nc.tensor.matmul(pm2[:, :nn], lhsT=AA1t[:], rhs=xhT[:, :nn],
                                     start=True, stop=True)
                    aaT = pa.tile([16, 512], F32, tag="aaT")
                    nc.vector.tensor_copy(out=aaT[:, :nn], in_=pm2[:, :nn])
                    ptx = ppa.tile([128, 512], F32, tag="ppx")
                    pt2 = ppa.tile([128, 64], F32, tag="pp3")
                    grps = ng + (1 if tail else 0)
                    for g in range(grps):
                        s = g * 128
                        w_ = min(128, nn - s)
                        nc.tensor.transpose(ptx[:w_, s:s + 128],
                                            xhT[:, s:s + w_], IDENT[:])
                        nc.tensor.transpose(pt2[:w_, g * 16:(g + 1) * 16],
                                            aaT[:, s:s + w_], IDENT[:16, :16])
                    xh = pa.tile([128, 512], TD, tag="xh")
                    nc.vector.tensor_copy(out=xh[:], in_=ptx[:])
                    aa = pa.tile([128, 512], TD, tag="aa")
                    nc.vector.memset(aa[:], 0.0)
                    nc.vector.tensor_copy(
                        out=aa[:].rearrange("n (g w) -> n g w",
                                            w=128)[:, :grps, 0:16],
                        in_=pt2[:, 0:grps * 16].rearrange(
                            "n (g w) -> n g w", w=16))
                    if nn == 512:
                        nc.sync.dma_start(
                            out=P1L[c0:c0 + 512, 0:128].rearrange(
                                "(g n) w -> n g w", n=128),
                            in_=xh[:].rearrange("n (g w) -> n g w", w=128))
                        nc.sync.dma_start(
                            out=P1L[c0:c0 + 512, 128:P1W].rearrange(
                                "(g n) w -> n g w", n=128),
                            in_=aa[:].rearrange("n (g w) -> n g w", w=128))
                    else:
                        for g in range(grps):
                            s = g * 128
                            w_ = min(128, nn - s)
                            nc.sync.dma_start(
                                out=P1L[c0 + s:c0 + s + w_, 0:128],
                                in_=xh[:w_, s:s + 128])
                            nc.sync.dma_start(
                                out=P1L[c0 + s:c0 + s + w_, 128:P1W],
                                in_=aa[:w_, s:s + 128])

            with nc.named_scope("ag1"):
                if not SKIP_AG:
                    nc.gpsimd.collective_compute(
                        "AllGather", mm.bypass, replica_groups=RG,
                        ins=[P1L[:, :]], outs=[P1F[:, :]])

            # ---------------- L1 edge stage (+ fused L2 node stage) ----------------
            with nc.named_scope("edge1"), \
                 tc.tile_pool(name="gma", bufs=2) as gmap, \
                 tc.tile_pool(name="gmb", bufs=2) as gmbp, \
                 tc.tile_pool(name="gad", bufs=2) as gadp, \
                 tc.tile_pool(name="off", bufs=2) as offp, \
                 tc.tile_pool(name="sml", bufs=3) as sml, \
                 tc.tile_pool(name="sal", bufs=2) as salp, \
                 tc.tile_pool(name="hb", bufs=1) as hbp, \
                 tc.tile_pool(name="a2", bufs=1) as a2p, \
                 tc.tile_pool(name="ps1", bufs=2, space="PSUM") as ps1p, \
                 tc.tile_pool(name="psa2", bufs=1, space="PSUM") as psa2p:

                for sb in ([] if SKIP_EDGE else range(NSB)):
                    ix = offp.tile([128, CW + G * K], I16, tag="ix")
                    nc.sync.dma_start(out=ix[:], in_=IDX[sb, :, :])
                    dl = ix[:, CW:CW + G * K].bitcast(TD)
                    gmA = gmap.tile([128, G * Ka * P1W], TD, tag="gmA")
                    gmA3 = gmA[:].rearrange("p (r w) -> p r w", w=P1W)
                    gmA4 = gmA[:].rearrange("p (g k w) -> p g k w", g=G, w=P1W)
                    gcalls(ix, gmA3, P1F[0:HALF, :], 0, G * Ka, P1W)
                    gmB = gmbp.tile([128, G * Kb * P1W], TD, tag="gmB")
                    gmB3 = gmB[:].rearrange("p (r w) -> p r w", w=P1W)
                    gmB4 = gmB[:].rearrange("p (g k w) -> p g k w", g=G, w=P1W)
                    gcalls(ix, gmB3, P1F[HALF:n_nodes, :], G * Ka, G * Kb, P1W)
                    gad = gadp.tile([128, G * K * (P1W - 128)], TD, tag="gad")
                    gad3 = gad[:].rearrange("p (r w) -> p r w", w=P1W - 128)
                    gad4 = gad[:].rearrange("p (g k w) -> p g k w", g=G,
                                            w=P1W - 128)
                    gcalls(ix, gad3, P1L[:, 128:P1W], G * (Ka + Kb), G * K,
                           P1W - 128, estep=P1W)

                    # whole-super-block score pipeline (a few fat instructions)
                    eab = sml.tile([128, G * K * 8], F32, tag="eab")
                    eaA = eab[:, 0:G * Ka * 8]
                    eaA4 = eaA.rearrange("p (g k h) -> p g k h", g=G, h=8)
                    nc.vector.tensor_tensor(
                        out=eaA4, in0=gmA4[:, :, :, 128:136],
                        in1=gad4[:, :, 0:Ka, 8:16], op=mm.add)
                    eaB = eab[:, G * Ka * 8:]
                    eaB4 = eaB.rearrange("p (g k h) -> p g k h", g=G, h=8)
                    nc.vector.tensor_tensor(
                        out=eaB4, in0=gmB4[:, :, :, 128:136],
                        in1=gad4[:, :, Ka:K, 8:16], op=mm.add)
                    tl = sml.tile([128, G * K * 8], F32, tag="tl")
                    nc.vector.tensor_scalar_mul(tl[:], eab[:], NEG_SLOPE)
                    nc.vector.tensor_tensor(out=eab[:], in0=eab[:], in1=tl[:],
                                            op=mm.max)
                    nc.scalar.activation(out=gmA4[:, :, :, 128:136], in_=eaA4,
                                         func=ACT.Exp)
                    nc.scalar.activation(out=gmB4[:, :, :, 128:136], in_=eaB4,
                                         func=ACT.Exp)
                    # msg = xh * ex, whole super-block per half
                    nc.vector.tensor_tensor(
                        out=gmA3[:, :, 0:128].rearrange(
                            "p r (h c) -> p r h c", c=16),
                        in0=gmA3[:, :, 0:128].rearrange(
                            "p r (h c) -> p r h c", c=16),
                        in1=gmA3[:, :, 128:136].rearrange(
                            "p r (h o) -> p r h o", o=1).to_broadcast(
                            [128, G * Ka, 8, 16]),
                        op=mm.mult)
                    nc.vector.tensor_tensor(
                        out=gmB3[:, :, 0:128].rearrange(
                            "p r (h c) -> p r h c", c=16),
                        in0=gmB3[:, :, 0:128].rearrange(
                            "p r (h c) -> p r h c", c=16),
                        in1=gmB3[:, :, 128:136].rearrange(
                            "p r (h o) -> p r h o", o=1).to_broadcast(
                            [128, G * Kb, 8, 16]),
                        op=mm.mult)

                    hball = hbp.tile([128, G * 136], F32, tag="hball")
                    hb4 = hball[:].rearrange("p (g w) -> p g w", g=G)
                    for bi in range(G):
                        S_all = salp.tile([128, K * 128], TD, tag="sall")
                        nc.vector.tensor_tensor(
                            out=S_all[:].rearrange("p (k e) -> p k e", e=128),
                            in0=IOTAt[:].rearrange(
                                "p (o e) -> p o e", o=1).to_broadcast(
                                [128, K, 128]),
                            in1=dl[:, bi * K:(bi + 1) * K].rearrange(
                                "p (k o) -> p k o", o=1).to_broadcast(
                                [128, K, 128]),
                            op=mm.is_equal)
                        ps = ps1p.tile([128, 136], F32, tag="ps")
                        for k in range(K):
                            if k < Ka:
                                r = bi * Ka + k
                                ck = gmA[:, r * P1W:r * P1W + 136]
                            else:
                                r = bi * Kb + (k - Ka)
                                ck = gmB[:, r * P1W:r * P1W + 136]
                            nc.tensor.matmul(
                                ps[:], lhsT=S_all[:, k * 128:(k + 1) * 128],
                                rhs=ck, start=(k == 0), stop=(k == K - 1))
                        nc.vector.tensor_copy(
                            out=hball[:, bi * 136:(bi + 1) * 136], in_=ps[:])
                    # batched epilogue over the super-block:
                    # h = msg/den + b1 ; elu  (in place over hb4[:, :, 0:128])
                    rd = sml.tile([128, G * 8], F32, tag="rd")
                    rd3 = rd[:].rearrange("p (g h) -> p g h", g=G)
                    nc.vector.tensor_scalar_add(rd3, hb4[:, :, 128:136], EPS)
                    nc.vector.reciprocal(rd[:], rd[:])
                    hm = hbp.tile([128, G * 128], F32, tag="hm")
                    hm4 = hm[:].rearrange("p (g h c) -> p g h c", g=G, c=16)
                    nc.vector.tensor_tensor(
                        out=hm4,
                        in0=hb4[:, :, 0:128].rearrange(
                            "p g (h c) -> p g h c", c=16),
                        in1=rd3.rearrange("p g (h o) -> p g h o",
                                          o=1).to_broadcast([128, G, 8, 16]),
                        op=mm.mult)
                    nc.vector.tensor_tensor(
                        out=hm[:].rearrange("p (g w) -> p g w", g=G),
                        in0=hm[:].rearrange("p (g w) -> p g w", g=G),
                        in1=B1t[:].rearrange("p (o w) -> p o w",
                                             o=1).to_broadcast([128, G, 128]),
                        op=mm.add)
                    tm = hbp.tile([128, G * 128], F32, tag="tm")
                    nc.vector.tensor_scalar_min(tm[:], hm[:], 0.0)
                    nc.scalar.activation(out=tm[:], in_=tm[:], func=ACT.Exp)
                    nc.vector.tensor_scalar_sub(tm[:], tm[:], 1.0)
                    nc.vector.tensor_tensor(out=hm[:], in0=hm[:], in1=tm[:],
                                            op=mm.max)
                    # batched L2 node stage: hT (packed valid slots), then
                    # x2 = W2^T @ hT and a2 = AA2^T @ x2T, written to P2L by
                    # transposing DMAs
                    hTall = a2p.tile([128, G * BLK], F32, tag="hTall")
                    for bi in range(G):
                        pt = psa2p.tile([128, 128], F32, tag="pa2")
                        nc.tensor.transpose(pt[:, :BLK],
                                            hm[:BLK, bi * 128:(bi + 1) * 128],
                                            IDENT[:BLK, :BLK])
                        nc.vector.tensor_copy(
                            out=hTall[:, bi * BLK:(bi + 1) * BLK],
                            in_=pt[:, :BLK])
                    x2T = a2p.tile([64, G * BLK], F32, tag="x2T")
                    for c0 in range(0, G * BLK, 512):
                        c1 = min(c0 + 512, G * BLK)
                        p2m = psa2p.tile([64, 512], F32, tag="p2m")
                        nc.tensor.matmul(p2m[:, :c1 - c0], lhsT=W2t[:],
                                         rhs=hTall[:, c0:c1],
                                         start=True, stop=True)
                        nc.vector.tensor_copy(out=x2T[:, c0:c1],
                                              in_=p2m[:, :c1 - c0])
                    a2T = a2p.tile([2, G * BLK], F32, tag="a2T")
                    for c0 in range(0, G * BLK, 512):
                        c1 = min(c0 + 512, G * BLK)
                        p2a = psa2p.tile([2, 512], F32, tag="p2a")
                        nc.tensor.matmul(p2a[:, :c1 - c0], lhsT=AA2t[:],
                                         rhs=x2T[:, c0:c1],
                                         start=True, stop=True)
                        nc.vector.tensor_copy(out=a2T[:, c0:c1],
                                              in_=p2a[:, :c1 - c0])
                    x2r = a2p.tile([128, G * 64], TD, tag="x2r")
                    a2r = a2p.tile([128, G * 2], TD, tag="a2r")
                    for bi in range(G):
                        p2t = psa2p.tile([128, 64], F32, tag="p2t")
                        nc.tensor.transpose(p2t[:BLK, :],
                                            x2T[:, bi * BLK:(bi + 1) * BLK],
                                            IDENT[:64, :64])
                        nc.vector.tensor_copy(
                            out=x2r[:BLK, bi * 64:(bi + 1) * 64],
                            in_=p2t[:BLK, :])
                        p2u = psa2p.tile([128, 2], F32, tag="p2u")
                        nc.tensor.transpose(p2u[:BLK, :],
                                            a2T[:, bi * BLK:(bi + 1) * BLK],
                                            IDENT[:2, :2])
                        nc.vector.tensor_copy(
                            out=a2r[:BLK, bi * 2:(bi + 1) * 2],
                            in_=p2u[:BLK, :])
                    n0 = sb * G * BLK
                    nc.sync.dma_start(
                        out=P2L[n0:n0 + G * BLK, 0:64].rearrange(
                            "(g n) w -> n g w", n=BLK),
                        in_=x2r[:BLK, :].rearrange("n (g w) -> n g w", g=G))
                    nc.sync.dma_start(
                        out=P2L[n0:n0 + G * BLK, 64:66].rearrange(
                            "(g n) w -> n g w", n=BLK),
                        in_=a2r[:BLK, :].rearrange("n (g w) -> n g w", g=G))

            with nc.named_scope("ag2"):
                if not SKIP_AG:
                    nc.gpsimd.collective_compute(
                        "AllGather", mm.bypass, replica_groups=RG,
                        ins=[P2L[:, :]], outs=[P2F[:, :]])

            # ---------------- L2 edge stage ----------------
            with nc.named_scope("edge2"), \
                 tc.tile_pool(name="gma2", bufs=2) as gmap2, \
                 tc.tile_pool(name="gmb2", bufs=2) as gmbp2, \
                 tc.tile_pool(name="gad2", bufs=2) as gadp2, \
                 tc.tile_pool(name="off2", bufs=2) as offp2, \
                 tc.tile_pool(name="sml2", bufs=3) as sml2, \
                 tc.tile_pool(name="sal2", bufs=2) as salp2, \
                 tc.tile_pool(name="ob", bufs=2) as obp, \
                 tc.tile_pool(name="ps2", bufs=2, space="PSUM") as ps2p:

                for sb in ([] if SKIP_EDGE else range(NSB)):
                    ix = offp2.tile([128, CW + G * K], I16, tag="ix2")
                    nc.sync.dma_start(out=ix[:], in_=IDX[sb, :, :])
                    dl = ix[:, CW:CW + G * K].bitcast(TD)
                    gmA = gmap2.tile([128, G * Ka * P2W], TD, tag="gmA2")
                    gmA3 = gmA[:].rearrange("p (r w) -> p r w", w=P2W)
                    gmA4 = gmA[:].rearrange("p (g k w) -> p g k w", g=G, w=P2W)
                    gcalls(ix, gmA3, P2F[0:HALF, :], 0, G * Ka, P2W)
                    gmB = gmbp2.tile([128, G * Kb * P2W], TD, tag="gmB2")
                    gmB3 = gmB[:].rearrange("p (r w) -> p r w", w=P2W)
                    gmB4 = gmB[:].rearrange("p (g k w) -> p g k w", g=G, w=P2W)
                    gcalls(ix, gmB3, P2F[HALF:n_nodes, :], G * Ka, G * Kb, P2W)
                    gad = gadp2.tile([128, G * K * P2W], TD, tag="gad2")
                    gad3 = gad[:].rearrange("p (r w) -> p r w", w=P2W)
                    gad4 = gad[:].rearrange("p (g k w) -> p g k w", g=G, w=P2W)
                    gcalls(ix, gad3, P2L[:, :], G * (Ka + Kb), G * K, P2W)

                    eab = sml2.tile([128, G * K], F32, tag="eab2")
                    eaA = eab[:, 0:G * Ka]
                    eaA3 = eaA.rearrange("p (g k) -> p g k", g=G)
                    nc.vector.tensor_tensor(
                        out=eaA3,
                        in0=gmA4[:, :, :, 64:65].rearrange("p g k o -> p g (k o)"),
                        in1=gad4[:, :, 0:Ka, 65:66].rearrange("p g k o -> p g (k o)"),
                        op=mm.add)
                    eaB = eab[:, G * Ka:]
                    eaB3 = eaB.rearrange("p (g k) -> p g k", g=G)
                    nc.vector.tensor_tensor(
                        out=eaB3,
                        in0=gmB4[:, :, :, 64:65].rearrange("p g k o -> p g (k o)"),
                        in1=gad4[:, :, Ka:K, 65:66].rearrange("p g k o -> p g (k o)"),
                        op=mm.add)
                    tl = sml2.tile([128, G * K], F32, tag="tl2")
                    nc.vector.tensor_scalar_mul(tl[:], eab[:], NEG_SLOPE)
                    nc.vector.tensor_tensor(out=eab[:], in0=eab[:], in1=tl[:],
                                            op=mm.max)
                    nc.scalar.activation(
                        out=gmA4[:, :, :, 64:65].rearrange("p g k o -> p g (k o)"),
                        in_=eaA3, func=ACT.Exp)
                    nc.scalar.activation(
                        out=gmB4[:, :, :, 64:65].rearrange("p g k o -> p g (k o)"),
                        in_=eaB3, func=ACT.Exp)
                    nc.vector.tensor_tensor(
                        out=gmA3[:, :, 0:64], in0=gmA3[:, :, 0:64],
                        in1=gmA3[:, :, 64:65].to_broadcast([128, G * Ka, 64]),
                        op=mm.mult)
                    nc.vector.tensor_tensor(
                        out=gmB3[:, :, 0:64], in0=gmB3[:, :, 0:64],
                        in1=gmB3[:, :, 64:65].to_broadcast([128, G * Kb, 64]),
                        op=mm.mult)

                    oball = obp.tile([128, G * 65], F32, tag="oball")
                    ob4 = oball[:].rearrange("p (g w) -> p g w", g=G)
                    for bi in range(G):
                        S_all = salp2.tile([128, K * 128], TD, tag="sall2")
                        nc.vector.tensor_tensor(
                            out=S_all[:].rearrange("p (k e) -> p k e", e=128),
                            in0=IOTAt[:].rearrange(
                                "p (o e) -> p o e", o=1).to_broadcast(
                                [128, K, 128]),
                            in1=dl[:, bi * K:(bi + 1) * K].rearrange(
                                "p (k o) -> p k o", o=1).to_broadcast(
                                [128, K, 128]),
                            op=mm.is_equal)
                        ps = ps2p.tile([128, 65], F32, tag="psb")
                        for k in range(K):
                            if k < Ka:
                                r = bi * Ka + k
                                ck = gmA[:, r * P2W:r * P2W + 65]
                            else:
                                r = bi * Kb + (k - Ka)
                                ck = gmB[:, r * P2W:r * P2W + 65]
                            nc.tensor.matmul(
                                ps[:], lhsT=S_all[:, k * 128:(k + 1) * 128],
                                rhs=ck, start=(k == 0), stop=(k == K - 1))
                        nc.vector.tensor_copy(
                            out=oball[:, bi * 65:(bi + 1) * 65], in_=ps[:])
                    rd = sml2.tile([128, G], F32, tag="rd2")
                    rd3 = rd[:].rearrange("p (g o) -> p g o", o=1)
                    nc.vector.tensor_scalar_add(rd3, ob4[:, :, 64:65], EPS)
                    nc.vector.reciprocal(rd[:], rd[:])
                    ob = obp.tile([128, G * 64], F32, tag="ob")
                    ob3 = ob[:].rearrange("p (g w) -> p g w", g=G)
                    nc.vector.tensor_tensor(
                        out=ob3, in0=ob4[:, :, 0:64],
                        in1=rd3.to_broadcast([128, G, 64]), op=mm.mult)
                    nc.vector.tensor_tensor(
                        out=ob3, in0=ob3,
                        in1=B2t[:].rearrange("p (o w) -> p o w",
                                             o=1).to_broadcast([128, G, 64]),
                        op=mm.add)
                    n0 = sb * G * BLK
                    nc.sync.dma_start(
                        out=OUT[n0:n0 + G * BLK, :].rearrange(
                            "(g n) w -> n g w", n=BLK),
                        in_=ob[:BLK, :].rearrange("n (g w) -> n g w", g=G))

    nc.compile()
    return nc


def _run(inputs, sim=False, trace=False):
    in_maps, prm = _host_prep(**inputs)
    nc = _build_program(prm)
    n_cores = prm["n_cores"]
    if sim:
        from concourse.bass_interp import MultiCoreSim
        ms = MultiCoreSim(nc, num_cores=n_cores)
        for c in range(n_cores):
            for k, v in in_maps[c].items():
                ms.cores[c].tensor(k)[:] = v
        ms.simulate()
        outs = [np.array(ms.cores[c].tensor("out")) for c in range(n_cores)]
        got = np.concatenate(outs, axis=0)
        res = None
    else:
        from concourse.bass_utils import run_bass_kernel_spmd
        res = run_bass_kernel_spmd(nc, in_maps, core_ids=list(range(n_cores)),
                                   trace=trace)
        outs = [res.results[c]["out"] for c in range(n_cores)]
        got = np.concatenate(outs, axis=0)
    full = np.empty_like(got)
    full[prm["perm"]] = got
    return full, res


def kernel(**inputs):
    out, _ = _run({k: np.asarray(v) for k, v in inputs.items()})
    return out


# revision 47
# speedup vs baseline: 3.8073x; 1.0182x over previous
"""BiGAT (2-layer GAT, PyG-style with self-loops) on 8 Trainium2 NeuronCores.

Strategy: partition nodes (and their incoming edges) by destination across 8
cores. Nodes are permuted so every 125-node dst block carries a near-equal
edge count in both src-halves (two-pass balancing), making the per-block
chunk count uniform and small. Edges are sorted by dst and padded to a
uniform blocks-x-chunks structure so one SPMD program serves all cores.

The measured runtime cost structure on this stack is ~100us PER INSTRUCTION
(dispatch-bound; payload size nearly free), so the kernel minimizes the
instruction count:
  - gathers batched into maximal 1024-idx calls spanning G=5 blocks;
  - per-edge score/softmax vector work merged into a handful of whole-
    super-block instructions (strided multi-dim access patterns);
  - the one-hot scatter matrices for a block are built by a single
    is_equal; gathered rows are packed [xh | a_src | a_dst | pad] so the
    exp() output can overwrite a_src in place and each chunk's scatter is
    ONE PSUM-accumulating matmul over [msg | ex].

Per layer:
  node stage : xh = x @ W and attention dots (PE); packed rows written to a
               local DRAM table; AllGather replicates the table.
  edge stage : dma_gather rows by src and [a|..] rows by dst-local;
               e = lrelu(a_src+a_dst); ex = exp(e) (softmax max-shift
               skipped -- scores are O(10) so exp cannot overflow, and
               softmax is shift-invariant); msg = xh_src * ex; one-hot
               matmul scatter-adds [msg | ex] into PSUM per block; epilogue
               divides by the summed ex, adds bias. The L2 node stage is
               fused into the L1 edge epilogue.

dma_gather constraints honored: int16 indices (src tables split into two
<=25000-row halves; dst uses core-local indices), <=1024 idxs per call,
row strides and elem sizes multiples of 256B, indices wrapped [16, n/16]
and replicated to 128 partitions.
"""
import sys

sys.path.insert(0, "/opt/trn_rl_repo")

import heapq
import numpy as np

from concourse import bass, mybir
import concourse.bacc as bacc
import concourse.tile as tile
from concourse.masks import make_identity

F32 = mybir.dt.float32
I16 = mybir.dt.int16
TD = mybir.dt.bfloat16
import ml_dtypes
TNP = ml_dtypes.bfloat16

# ---------------- problem constants (hardcoded per contract) ----------------
N_NODES = 50000
N_EDGES = 800000
IN_C, HID_C, OUT_C, HEADS = 128, 16, 64, 8
NEG_SLOPE = 0.2
N_CORES = 8

# ---------------- sharding / tiling parameters ----------------
BLK = 125       # dst nodes per edge-stage block (<=128 for one-hot)
P1W = 256       # L1 table row: [xh(128) | a_src(8) | a_dst(8) | pad]
P2W = 128       # L2 table row: [x2(64) | a_src2(1) | a_dst2(1) | pad]
G = 5           # blocks per super-block (gather batching unit)
MAXI = 1024     # max idxs per dma_gather call (HW cap; >1024 crashes)
EPS = 1e-16


def _wrap16(idx):
    """[L] int array -> dma_gather wrapped layout [128, L//16] int16."""
    L = len(idx)
    w = idx.reshape(L // 16, 16).T
    return np.tile(w, (8, 1)).astype(np.int16)


def _balance_blocks2(src0, dst0, n_nodes, nblk_tot):
    """Two-pass node->block assignment: pass 1 equalizes total in-degree
    (fixing each node's half); pass 2 reassigns within each half to equalize
    per-block edge counts from BOTH halves. Returns perm (new pos -> node)."""
    HALF = n_nodes // 2
    deg = np.bincount(dst0, minlength=n_nodes)
    order = np.argsort(-deg, kind="stable")
    fill = np.zeros(nblk_tot, np.int32)
    perm = np.empty(n_nodes, np.int64)
    heap = [(0, b) for b in range(nblk_tot)]
    heapq.heapify(heap)
    for node in order:
        load, b = heapq.heappop(heap)
        perm[b * BLK + fill[b]] = node
        fill[b] += 1
        load += int(deg[node])
        if fill[b] < BLK:
            heapq.heappush(heap, (load, b))

    old2new = np.empty(n_nodes, np.int64)
    old2new[perm] = np.arange(n_nodes)
    src_in_a = old2new[src0] < HALF
    deg_a = np.bincount(dst0, weights=src_in_a.astype(np.float64),
                        minlength=n_nodes).astype(np.int64)
    deg_b = deg - deg_a

    perm2 = np.empty(n_nodes, np.int64)
    nb2 = nblk_tot // 2
    for half in (0, 1):
        nodes = perm[half * HALF:(half + 1) * HALF]
        o = np.argsort(-(deg_a[nodes] + deg_b[nodes]), kind="stable")
        nodes = nodes[o]
        la = np.zeros(nb2, np.int64)
        lb = np.zeros(nb2, np.int64)
        fl = np.zeros(nb2, np.int32)
        for node in nodes:
            da, db = deg_a[node], deg_b[node]
            cand = np.maximum(la + da, lb + db).astype(np.float64)
            cand[fl >= BLK] = np.inf
            b = int(np.argmin(cand))
            perm2[half * HALF + b * BLK + fl[b]] = node
            fl[b] += 1
            la[b] += da
            lb[b] += db
    # repair pass: swap nodes between over-cap and low-load blocks until both
    # per-block half-counts fit in 9 chunks
    CAP = 9 * 128
    la = np.zeros(nblk_tot, np.int64)
    lb = np.zeros(nblk_tot, np.int64)
    for b in range(nblk_tot):
        nodes = perm2[b * BLK:(b + 1) * BLK]
        la[b] = deg_a[nodes].sum()
        lb[b] = deg_b[nodes].sum()
    for half in (0, 1):
        bs = np.arange(half * nb2, (half + 1) * nb2)
        for _ in range(3000):
            w = bs[int(np.argmax(np.maximum(la[bs], lb[bs])))]
            if max(la[w], lb[w]) <= CAP:
                break
            dim_a = la[w] >= lb[w]
            key = deg_a if dim_a else deg_b
            wn = perm2[w * BLK:(w + 1) * BLK]
            iw = int(np.argmax(key[wn]))
            nw = wn[iw]
            order = np.argsort(la[bs] if dim_a else lb[bs])
            done = False
            for d in bs[order[:20]]:
                dn = perm2[d * BLK:(d + 1) * BLK]
                idn = int(np.argmin(key[dn]))
                nd = dn[idn]
                dla = deg_a[nw] - deg_a[nd]
                dlb = deg_b[nw] - deg_b[nd]
                if ((dla if dim_a else dlb) > 0 and la[d] + dla <= CAP
                        and lb[d] + dlb <= CAP):
                    perm2[w * BLK + iw], perm2[d * BLK + idn] = nd, nw
                    la[w] -= dla
                    lb[w] -= dlb
                    la[d] += dla
                    lb[d] += dlb
                    done = True
                    break
            if not done:
                break
    return perm2


def _host_prep(x, edge_index, W1, att_src1, att_dst1, b1, W2, att_src2, att_dst2, b2,
               n_nodes=N_NODES, n_cores=N_CORES):
    """Sort/pad edges, build per-core input maps and compile-time params."""
    NP = n_nodes // n_cores
    NB = NP // BLK
    assert NB * BLK == NP and NB % G == 0
    NSB = NB // G
    HALF = n_nodes // 2
    assert HALF < 32768 and NP < 32768

    src0 = np.concatenate([np.asarray(edge_index[0]), np.arange(n_nodes)]).astype(np.int64)
    dst0 = np.concatenate([np.asarray(edge_index[1]), np.arange(n_nodes)]).astype(np.int64)

    nblk_tot = n_cores * NB
    perm = _balance_blocks2(src0, dst0, n_nodes, nblk_tot)  # new pos -> old node
    old2new = np.empty(n_nodes, np.int64)
    old2new[perm] = np.arange(n_nodes)

    src = old2new[src0]
    dst = old2new[dst0]
    order = np.argsort(dst, kind="stable")
    src, dst = src[order], dst[order]

    blk_of = dst // BLK
    # within each dst-block, put src<HALF ("a") edges first
    order2 = np.lexsort((src >= HALF, blk_of))
    src, dst = src[order2], dst[order2]
    is_b = src >= HALF
    cnt_a = np.bincount(blk_of[order2], weights=~is_b, minlength=nblk_tot).astype(np.int64)
    cnt_b = np.bincount(blk_of[order2], weights=is_b, minlength=nblk_tot).astype(np.int64)
    starts = np.concatenate([[0], np.cumsum(cnt_a + cnt_b)]).astype(np.int64)
    Ka = int(np.ceil(cnt_a.max() / 128))
    Kb = int(np.ceil(cnt_b.max() / 128))
    K = Ka + Kb

    # per-block padded arrays in [a-pad | b-pad] chunk order
    srcA = np.zeros((nblk_tot, Ka * 128), np.int64)      # pad -> row 0
    srcB = np.zeros((nblk_tot, Kb * 128), np.int64)
    dstL = np.zeros((nblk_tot, K * 128), np.int64)       # dst local to core
    dloc = np.full((nblk_tot, K * 128), 999.0, np.float32)  # dst local to block
    for b in range(nblk_tot):
        na, nb_ = int(cnt_a[b]), int(cnt_b[b])
        s = starts[b]
        core = b // NB
        srcA[b, :na] = src[s:s + na]
        srcB[b, :nb_] = src[s + na:s + na + nb_] - HALF
        dstL[b, :na] = dst[s:s + na] - core * NP
        dstL[b, Ka * 128:Ka * 128 + nb_] = dst[s + na:s + na + nb_] - core * NP
        dloc[b, :na] = dst[s:s + na] - b * BLK
        dloc[b, Ka * 128:Ka * 128 + nb_] = dst[s + na:s + na + nb_] - b * BLK

    # shared (replicated) weights
    AA1 = np.zeros((128, 16), np.float32)
    asrc1 = np.asarray(att_src1, np.float32)
    adst1 = np.asarray(att_dst1, np.float32)
    for h in range(HEADS):
        AA1[16 * h:16 * (h + 1), h] = asrc1[h]
        AA1[16 * h:16 * (h + 1), 8 + h] = adst1[h]
    AA2 = np.stack([np.asarray(att_src2, np.float32)[0],
                    np.asarray(att_dst2, np.float32)[0]], axis=1)  # [64, 2]
    shared = {
        "W1": np.asarray(W1, np.float32),
        "AA1": AA1,
        "B1": np.tile(np.asarray(b1, np.float32), (128, 1)),
        "W2": np.asarray(W2, np.float32),
        "AA2": AA2,
        "B2": np.tile(np.asarray(b2, np.float32), (128, 1)),
        "IOTA": np.tile(np.arange(128), (128, 1)).astype(TNP),
    }

    xT = np.ascontiguousarray(np.asarray(x, np.float32).T)  # [128, N] (old order)

    in_maps = []
    for c in range(n_cores):
        lo = c * NB
        # super-block idx layout: [A(b0..) | B(b0..) | dst(b0..)] wrapped
        idx = np.stack([
            np.concatenate(
                [_wrap16(srcA[lo + s * G + g]) for g in range(G)] +
                [_wrap16(srcB[lo + s * G + g]) for g in range(G)] +
                [_wrap16(dstL[lo + s * G + g]) for g in range(G)], axis=1)
            for s in range(NSB)])
        dl = np.stack([
            np.concatenate(
                [dloc[lo + s * G + g].reshape(K, 128).T for g in range(G)], axis=1)
            for s in range(NSB)])                        # [NSB, 128, G*K]
        dl_i16 = dl.astype(TNP).view(np.int16)
        m = dict(shared)
        m["xT"] = np.ascontiguousarray(xT[:, perm[c * NP:(c + 1) * NP]])
        m["IDX"] = np.ascontiguousarray(
            np.concatenate([idx, dl_i16], axis=2))
        in_maps.append(m)

    prm = dict(NP=NP, NB=NB, NSB=NSB, K=K, Ka=Ka, Kb=Kb,
               n_nodes=n_nodes, n_cores=n_cores, HALF=HALF, perm=perm)
    return in_maps, prm


def _build_program(prm, repeat=1):
    import os
    SKIP_AG = bool(int(os.environ.get("BG_SKIP_AG", "0")))
    SKIP_EDGE = bool(int(os.environ.get("BG_SKIP_EDGE", "0")))
    SKIP_NODE = bool(int(os.environ.get("BG_SKIP_NODE", "0")))
    NP, NSB, K, Ka, Kb = prm["NP"], prm["NSB"], prm["K"], prm["Ka"], prm["Kb"]
    HALF = prm["HALF"]
    n_nodes, n_cores = prm["n_nodes"], prm["n_cores"]
    RG = [list(range(n_cores))]
    CW = G * (Ka + Kb + K) * 8  # idx cols per super-block (+G*K dloc cols)

    nc = bacc.Bacc("TRN2", target_bir_lowering=False, debug=False,
                   num_devices=n_cores, num_swdge_queues=4)
    qn = [0]  # round-robin SWDGE queue assignment for gathers

    def next_q():
        qn[0] += 1
        return qn[0] % 4

    # inputs
    xT = nc.dram_tensor("xT", [128, NP], F32, kind="ExternalInput")
    W1 = nc.dram_tensor("W1", [128, 128], F32, kind="ExternalInput")
    AA1 = nc.dram_tensor("AA1", [128, 16], F32, kind="ExternalInput")
    B1 = nc.dram_tensor("B1", [128, 128], F32, kind="ExternalInput")
    W2 = nc.dram_tensor("W2", [128, 64], F32, kind="ExternalInput")
    AA2 = nc.dram_tensor("AA2", [64, 2], F32, kind="ExternalInput")
    B2 = nc.dram_tensor("B2", [128, 64], F32, kind="ExternalInput")
    IOTA = nc.dram_tensor("IOTA", [128, 128], TD, kind="ExternalInput")
    IDX = nc.dram_tensor("IDX", [NSB, 128, CW + G * K], I16,
                         kind="ExternalInput")
    OUT = nc.dram_tensor("out", [NP, OUT_C], F32, kind="ExternalOutput")
    # internal DRAM
    P1L = nc.dram_tensor("P1L", [NP, P1W], TD)
    P1F = nc.dram_tensor("P1F", [n_nodes, P1W], TD, addr_space="Shared")
    P2L = nc.dram_tensor("P2L", [NP, P2W], TD)
    P2F = nc.dram_tensor("P2F", [n_nodes, P2W], TD, addr_space="Shared")

    mm = mybir.AluOpType
    ACT = mybir.ActivationFunctionType

    def gcalls(ix, out3, table, col0, nchunk, elem, estep=None):
        step = MAXI // 128
        for c0 in range(0, nchunk, step):
            c1 = min(c0 + step, nchunk)
            nc.gpsimd.dma_gather(
                out_ap=out3[:, c0:c1, :], in_ap=table,
                idxs_ap=ix[:, (col0 + c0) * 8:(col0 + c1) * 8],
                num_idxs=(c1 - c0) * 128,
                num_idxs_reg=(c1 - c0) * 128, elem_size=elem,
                elem_step=estep, queue_num=next_q())

    from contextlib import ExitStack
    with tile.TileContext(nc) as tc, ExitStack() as ctx:
        cst = ctx.enter_context(tc.tile_pool(name="cst", bufs=1))
        W1t = cst.tile([128, 128], F32)
        AA1t = cst.tile([128, 16], F32)
        B1t = cst.tile([128, 128], F32)
        W2t = cst.tile([128, 64], F32)
        AA2t = cst.tile([64, 2], F32)
        B2t = cst.tile([128, 64], F32)
        IOTAt = cst.tile([128, 128], TD)
        IDENT = cst.tile([128, 128], F32)
        for t, d in ((W1t, W1), (AA1t, AA1), (B1t, B1), (W2t, W2),
                     (AA2t, AA2), (B2t, B2), (IOTAt, IOTA)):
            nc.sync.dma_start(out=t[:], in_=d[:, :])
        make_identity(nc, IDENT[:])

        # body may be repeated for differential benchmarking
        for _rep in range(repeat):
            # ---------------- stage A: L1 node stage (512-wide tiles) ----------------
            with nc.named_scope("nodeA"), \
                 tc.tile_pool(name="pa", bufs=2) as pa, \
                 tc.tile_pool(name="ppa", bufs=2, space="PSUM") as ppa:
                for c0 in ([] if SKIP_NODE else range(0, NP, 512)):
                    nn = min(512, NP - c0)
                    ng = nn // 128          # full 128-groups
                    tail = nn - ng * 128    # ragged remainder (last tile)
                    xt = pa.tile([128, 512], F32, tag="xt")
                    nc.sync.dma_start(out=xt[:, :nn], in_=xT[:, c0:c0 + nn])
                    pm = ppa.tile([128, 512], F32, tag="pp")
                    nc.tensor.matmul(pm[:, :nn], lhsT=W1t[:], rhs=xt[:, :nn],
                                     start=True, stop=True)
                    xhT = pa.tile([128, 512], F32, tag="xhT")
                    nc.vector.tensor_copy(out=xhT[:, :nn], in_=pm[:, :nn])
                    pm2 = ppa.tile([16, 512], F32, tag="pp2")
                    nc.tensor.matmul(pm2[:, :nn], lhsT=AA1t[:], rhs=xhT[:, :nn],
                                     start=True, stop=True)
                    aaT = pa.tile([16, 512], F32, tag="aaT")
                    nc.vector.tensor_copy(out=aaT[:, :nn], in_=pm2[:, :nn])
                    ptx = ppa.tile([128, 512], F32, tag="ppx")
                    pt2 = ppa.tile([128, 64], F32, tag="pp3")
                    grps = ng + (1 if tail else 0)
                    for g in range(grps):
                        s = g * 128
                        w_ = min(128, nn - s)
                        nc.tensor.transpose(ptx[:w_, s:s + 128],
                                            xhT[:, s:s + w_], IDENT[:])
                        nc.tensor.transpose(pt2[:w_, g * 16:(g + 1) * 16],
                                            aaT[:, s:s + w_], IDENT[:16, :16])
                    xh = pa.tile([128, 512], TD, tag="xh")
                    nc.vector.tensor_copy(out=xh[:], in_=ptx[:])
                    aa = pa.tile([128, 512], TD, tag="aa")
                    nc.vector.memset(aa[:], 0.0)
                    nc.vector.tensor_copy(
                        out=aa[:].rearrange("n (g w) -> n g w",
                                            w=128)[:, :grps, 0:16],
                        in_=pt2[:, 0:grps * 16].rearrange(
                            "n (g w) -> n g w", w=16))
                    if nn == 512:
                        nc.sync.dma_start(
                            out=P1L[c0:c0 + 512, 0:128].rearrange(
                                "(g n) w -> n g w", n=128),
                            in_=xh[:].rearrange("n (g w) -> n g w", w=128))
                        nc.sync.dma_start(
                            out=P1L[c0:c0 + 512, 128:P1W].rearrange(
                                "(g n) w -> n g w", n=128),
                            in_=aa[:].rearrange("n (g w) -> n g w", w=128))
                    else:
                        for g in range(grps):
                            s = g * 128
                            w_ = min(128, nn - s)
                            nc.sync.dma_start(
                                out=P1L[c0 + s:c0 + s + w_, 0:128],
                                in_=xh[:w_, s:s + 128])
                            nc.sync.dma_start(
                                out=P1L[c0 + s:c0 + s + w_, 128:P1W],
                                in_=aa[:w_, s:s + 128])

            with nc.named_scope("ag1"):
                if not SKIP_AG:
                    nc.gpsimd.collective_compute(
                        "AllGather", mm.bypass, replica_groups=RG,
                        ins=[P1L[:, :]], outs=[P1F[:, :]])

            # ---------------- L1 edge stage (+ fused L2 node stage) ----------------
            with nc.named_scope("edge1"), \
                 tc.tile_pool(name="gma", bufs=2) as gmap, \
                 tc.tile_pool(name="gmb", bufs=2) as gmbp, \
                 tc.tile_pool(name="gad", bufs=2) as gadp, \
                 tc.tile_pool(name="off", bufs=2) as offp, \
                 tc.tile_pool(name="sml", bufs=3) as sml, \
                 tc.tile_pool(name="sal", bufs=1) as salp, \
                 tc.tile_pool(name="hb", bufs=1) as hbp, \
                 tc.tile_pool(name="a2", bufs=1) as a2p, \
                 tc.tile_pool(name="ps1", bufs=2, space="PSUM") as ps1p, \
                 tc.tile_pool(name="psa2", bufs=1, space="PSUM") as psa2p:

                for sb in ([] if SKIP_EDGE else range(NSB)):
                    ix = offp.tile([128, CW + G * K], I16, tag="ix")
                    nc.sync.dma_start(out=ix[:], in_=IDX[sb, :, :])
                    dl = ix[:, CW:CW + G * K].bitcast(TD)
                    gmA = gmap.tile([128, G * Ka * P1W], TD, tag="gmA")
                    gmA3 = gmA[:].rearrange("p (r w) -> p r w", w=P1W)
                    gmA4 = gmA[:].rearrange("p (g k w) -> p g k w", g=G, w=P1W)
                    gcalls(ix, gmA3, P1F[0:HALF, :], 0, G * Ka, P1W)
                    gmB = gmbp.tile([128, G * Kb * P1W], TD, tag="gmB")
                    gmB3 = gmB[:].rearrange("p (r w) -> p r w", w=P1W)
                    gmB4 = gmB[:].rearrange("p (g k w) -> p g k w", g=G, w=P1W)
                    gcalls(ix, gmB3, P1F[HALF:n_nodes, :], G * Ka, G * Kb, P1W)
                    gad = gadp.tile([128, G * K * (P1W - 128)], TD, tag="gad")
                    gad3 = gad[:].rearrange("p (r w) -> p r w", w=P1W - 128)
                    gad4 = gad[:].rearrange("p (g k w) -> p g k w", g=G,
                                            w=P1W - 128)
                    gcalls(ix, gad3, P1L[:, 128:P1W], G * (Ka + Kb), G * K,
                           P1W - 128, estep=P1W)

                    # whole-super-block score pipeline (a few fat instructions)
                    eab = sml.tile([128, G * K * 8], F32, tag="eab")
                    eaA = eab[:, 0:G * Ka * 8]
                    eaA4 = eaA.rearrange("p (g k h) -> p g k h", g=G, h=8)
                    nc.vector.tensor_tensor(
                        out=eaA4, in0=gmA4[:, :, :, 128:136],
                        in1=gad4[:, :, 0:Ka, 8:16], op=mm.add)
                    eaB = eab[:, G * Ka * 8:]
                    eaB4 = eaB.rearrange("p (g k h) -> p g k h", g=G, h=8)
                    nc.vector.tensor_tensor(
                        out=eaB4, in0=gmB4[:, :, :, 128:136],
                        in1=gad4[:, :, Ka:K, 8:16], op=mm.add)
                    tl = sml.tile([128, G * K * 8], F32, tag="tl")
                    nc.vector.tensor_scalar_mul(tl[:], eab[:], NEG_SLOPE)
                    nc.vector.tensor_tensor(out=eab[:], in0=eab[:], in1=tl[:],
                                            op=mm.max)
                    nc.scalar.activation(out=gmA4[:, :, :, 128:136], in_=eaA4,
                                         func=ACT.Exp)
                    nc.scalar.activation(out=gmB4[:, :, :, 128:136], in_=eaB4,
                                         func=ACT.Exp)
                    # msg = xh * ex, whole super-block per half
                    nc.vector.tensor_tensor(
                        out=gmA3[:, :, 0:128].rearrange(
                            "p r (h c) -> p r h c", c=16),
                        in0=gmA3[:, :, 0:128].rearrange(
                            "p r (h c) -> p r h c", c=16),
                        in1=gmA3[:, :, 128:136].rearrange(
                            "p r (h o) -> p r h o", o=1).to_broadcast(
                            [128, G * Ka, 8, 16]),
                        op=mm.mult)
                    nc.vector.tensor_tensor(
                        out=gmB3[:, :, 0:128].rearrange(
                            "p r (h c) -> p r h c", c=16),
                        in0=gmB3[:, :, 0:128].rearrange(
                            "p r (h c) -> p r h c", c=16),
                        in1=gmB3[:, :, 128:136].rearrange(
                            "p r (h o) -> p r h o", o=1).to_broadcast(
                            [128, G * Kb, 8, 16]),
                        op=mm.mult)

                    hball = hbp.tile([128, G * 136], F32, tag="hball")
                    hb4 = hball[:].rearrange("p (g w) -> p g w", g=G)
                    S_sb = salp.tile([128, G * K * 128], TD, tag="ssb")
                    nc.vector.tensor_tensor(
                        out=S_sb[:].rearrange("p (q e) -> p q e", e=128),
                        in0=IOTAt[:].rearrange(
                            "p (o e) -> p o e", o=1).to_broadcast(
                            [128, G * K, 128]),
                        in1=dl[:, :].rearrange(
                            "p (q o) -> p q o", o=1).to_broadcast(
                            [128, G * K, 128]),
                        op=mm.is_equal)
                    for bi in range(G):
                        ps = ps1p.tile([128, 136], F32, tag="ps")
                        for k in range(K):
                            if k < Ka:
                                r = bi * Ka + k
                                ck = gmA[:, r * P1W:r * P1W + 136]
                            else:
                                r = bi * Kb + (k - Ka)
                                ck = gmB[:, r * P1W:r * P1W + 136]
                            q = bi * K + k
                            nc.tensor.matmul(
                                ps[:], lhsT=S_sb[:, q * 128:(q + 1) * 128],
                                rhs=ck, start=(k == 0), stop=(k == K - 1))
                        nc.vector.tensor_copy(
                            out=hball[:, bi * 136:(bi + 1) * 136], in_=ps[:])
                    # batched epilogue over the super-block:
                    # h = msg/den + b1 ; elu  (in place over hb4[:, :, 0:128])
                    rd = sml.tile([128, G * 8], F32, tag="rd")
                    rd3 = rd[:].rearrange("p (g h) -> p g h", g=G)
                    nc.vector.tensor_scalar_add(rd3, hb4[:, :, 128:136], EPS)
                    nc.vector.reciprocal(rd[:], rd[:])
                    hm = hbp.tile([128, G * 128], F32, tag="hm")
                    hm4 = hm[:].rearrange("p (g h c) -> p g h c", g=G, c=16)
                    nc.vector.tensor_tensor(
                        out=hm4,
                        in0=hb4[:, :, 0:128].rearrange(
                            "p g (h c) -> p g h c", c=16),
                        in1=rd3.rearrange("p g (h o) -> p g h o",
                                          o=1).to_broadcast([128, G, 8, 16]),
                        op=mm.mult)
                    nc.vector.tensor_tensor(
                        out=hm[:].rearrange("p (g w) -> p g w", g=G),
                        in0=hm[:].rearrange("p (g w) -> p g w", g=G),
                        in1=B1t[:].rearrange("p (o w) -> p o w",
                                             o=1).to_broadcast([128, G, 128]),
                        op=mm.add)
                    tm = hbp.tile([128, G * 128], F32, tag="tm")
                    nc.vector.tensor_scalar_min(tm[:], hm[:], 0.0)
                    nc.scalar.activation(out=tm[:], in_=tm[:], func=ACT.Exp)
                    nc.vector.tensor_scalar_sub(tm[:], tm[:], 1.0)
                    nc.vector.tensor_tensor(out=hm[:], in0=hm[:], in1=tm[:],
                                            op=mm.max)
                    # batched L2 node stage: hT (packed valid slots), then
                    # x2 = W2^T @ hT and a2 = AA2^T @ x2T, written to P2L by
                    # transposing DMAs
                    hTall = a2p.tile([128, G * BLK], F32, tag="hTall")
                    for bi in range(G):
                        pt = psa2p.tile([128, 128], F32, tag="pa2")
                        nc.tensor.transpose(pt[:, :BLK],
                                            hm[:BLK, bi * 128:(bi + 1) * 128],
                                            IDENT[:BLK, :BLK])
                        nc.vector.tensor_copy(
                            out=hTall[:, bi * BLK:(bi + 1) * BLK],
                            in_=pt[:, :BLK])
                    x2T = a2p.tile([64, G * BLK], F32, tag="x2T")
                    for c0 in range(0, G * BLK, 512):
                        c1 = min(c0 + 512, G * BLK)
                        p2m = psa2p.tile([64, 512], F32, tag="p2m")
                        nc.tensor.matmul(p2m[:, :c1 - c0], lhsT=W2t[:],
                                         rhs=hTall[:, c0:c1],
                                         start=True, stop=True)
                        nc.vector.tensor_copy(out=x2T[:, c0:c1],
                                              in_=p2m[:, :c1 - c0])
                    a2T = a2p.tile([2, G * BLK], F32, tag="a2T")
                    for c0 in range(0, G * BLK, 512):
                        c1 = min(c0 + 512, G * BLK)
                        p2a = psa2p.tile([2, 512], F32, tag="p2a")
                        nc.tensor.matmul(p2a[:, :c1 - c0], lhsT=AA2t[:],
                                         rhs=x2T[:, c0:c1],
                                         start=True, stop=True)
                        nc.vector.tensor_copy(out=a2T[:, c0:c1],
                                              in_=p2a[:, :c1 - c0])
                    x2r = a2p.tile([128, G * 64], TD, tag="x2r")
                    a2r = a2p.tile([128, G * 2], TD, tag="a2r")
                    for bi in range(G):
                        p2t = psa2p.tile([128, 64], F32, tag="p2t")
                        nc.tensor.transpose(p2t[:BLK, :],
                                            x2T[:, bi * BLK:(bi + 1) * BLK],
                                            IDENT[:64, :64])
                        nc.vector.tensor_copy(
                            out=x2r[:BLK, bi * 64:(bi + 1) * 64],
                            in_=p2t[:BLK, :])
                        p2u = psa2p.tile([128, 2], F32, tag="p2u")
                        nc.tensor.transpose(p2u[:BLK, :],
                                            a2T[:, bi * BLK:(bi + 1) * BLK],
                                            IDENT[:2, :2])
                        nc.vector.tensor_copy(
                            out=a2r[:BLK, bi * 2:(bi + 1) * 2],
                            in_=p2u[:BLK, :])
                    n0 = sb * G * BLK
                    nc.sync.dma_start(
                        out=P2L[n0:n0 + G * BLK, 0:64].rearrange(
                            "(g n) w -> n g w", n=BLK),
                        in_=x2r[:BLK, :].rearrange("n (g w) -> n g w", g=G))
                    nc.sync.dma_start(
                        out=P2L[n0:n0 + G * BLK, 64:66].rearrange(
                            "(g n) w -> n g w", n=BLK),
                        in_=a2r[:BLK, :].rearrange("n (g w) -> n g w", g=G))

            with nc.named_scope("ag2"):
                if not SKIP_AG:
                    nc.gpsimd.collective_compute(
                        "AllGather", mm.bypass, replica_groups=RG,
                        ins=[P2L[:, :]], outs=[P2F[:, :]])

            # ---------------- L2 edge stage ----------------
            with nc.named_scope("edge2"), \
                 tc.tile_pool(name="gma2", bufs=2) as gmap2, \
                 tc.tile_pool(name="gmb2", bufs=2) as gmbp2, \
                 tc.tile_pool(name="gad2", bufs=2) as gadp2, \
                 tc.tile_pool(name="off2", bufs=2) as offp2, \
                 tc.tile_pool(name="sml2", bufs=3) as sml2, \
                 tc.tile_pool(name="sal2", bufs=1) as salp2, \
                 tc.tile_pool(name="ob", bufs=2) as obp, \
                 tc.tile_pool(name="ps2", bufs=2, space="PSUM") as ps2p:

                for sb in ([] if SKIP_EDGE else range(NSB)):
                    ix = offp2.tile([128, CW + G * K], I16, tag="ix2")
                    nc.sync.dma_start(out=ix[:], in_=IDX[sb, :, :])
                    dl = ix[:, CW:CW + G * K].bitcast(TD)
                    gmA = gmap2.tile([128, G * Ka * P2W], TD, tag="gmA2")
                    gmA3 = gmA[:].rearrange("p (r w) -> p r w", w=P2W)
                    gmA4 = gmA[:].rearrange("p (g k w) -> p g k w", g=G, w=P2W)
                    gcalls(ix, gmA3, P2F[0:HALF, :], 0, G * Ka, P2W)
                    gmB = gmbp2.tile([128, G * Kb * P2W], TD, tag="gmB2")
                    gmB3 = gmB[:].rearrange("p (r w) -> p r w", w=P2W)
                    gmB4 = gmB[:].rearrange("p (g k w) -> p g k w", g=G, w=P2W)
                    gcalls(ix, gmB3, P2F[HALF:n_nodes, :], G * Ka, G * Kb, P2W)
                    gad = gadp2.tile([128, G * K * P2W], TD, tag="gad2")
                    gad3 = gad[:].rearrange("p (r w) -> p r w", w=P2W)
                    gad4 = gad[:].rearrange("p (g k w) -> p g k w", g=G, w=P2W)
                    gcalls(ix, gad3, P2L[:, :], G * (Ka + Kb), G * K, P2W)

                    eab = sml2.tile([128, G * K], F32, tag="eab2")
                    eaA = eab[:, 0:G * Ka]
                    eaA3 = eaA.rearrange("p (g k) -> p g k", g=G)
                    nc.vector.tensor_tensor(
                        out=eaA3,
                        in0=gmA4[:, :, :, 64:65].rearrange("p g k o -> p g (k o)"),
                        in1=gad4[:, :, 0:Ka, 65:66].rearrange("p g k o -> p g (k o)"),
                        op=mm.add)
                    eaB = eab[:, G * Ka:]
                    eaB3 = eaB.rearrange("p (g k) -> p g k", g=G)
                    nc.vector.tensor_tensor(
                        out=eaB3,
                        in0=gmB4[:, :, :, 64:65].rearrange("p g k o -> p g (k o)"),
                        in1=gad4[:, :, Ka:K, 65:66].rearrange("p g k o -> p g (k o)"),
                        op=mm.add)
                    tl = sml2.tile([128, G * K], F32, tag="tl2")
                    nc.vector.tensor_scalar_mul(tl[:], eab[:], NEG_SLOPE)
                    nc.vector.tensor_tensor(out=eab[:], in0=eab[:], in1=tl[:],
                                            op=mm.max)
                    nc.scalar.activation(
                        out=gmA4[:, :, :, 64:65].rearrange("p g k o -> p g (k o)"),
                        in_=eaA3, func=ACT.Exp)
                    nc.scalar.activation(
                        out=gmB4[:, :, :, 64:65].rearrange("p g k o -> p g (k o)"),
                        in_=eaB3, func=ACT.Exp)
                    nc.vector.tensor_tensor(
                        out=gmA3[:, :, 0:64], in0=gmA3[:, :, 0:64],
                        in1=gmA3[:, :, 64:65].to_broadcast([128, G * Ka, 64]),
                        op=mm.mult)
                    nc.vector.tensor_tensor(
                        out=gmB3[:, :, 0:64], in0=gmB3[:, :, 0:64],
                        in1=gmB3[:, :, 64:65].to_broadcast([128, G * Kb, 64]),
                        op=mm.mult)

                    oball = obp.tile([128, G * 65], F32, tag="oball")
                    ob4 = oball[:].rearrange("p (g w) -> p g w", g=G)
                    S_sb = salp2.tile([128, G * K * 128], TD, tag="ssb2")
                    nc.vector.tensor_tensor(
                        out=S_sb[:].rearrange("p (q e) -> p q e", e=128),
                        in0=IOTAt[:].rearrange(
                            "p (o e) -> p o e", o=1).to_broadcast(
                            [128, G * K, 128]),
                        in1=dl[:, :].rearrange(
                            "p (q o) -> p q o", o=1).to_broadcast(
                            [128, G * K, 128]),
                        op=mm.is_equal)
                    for bi in range(G):
                        ps = ps2p.tile([128, 65], F32, tag="psb")
                        for k in range(K):
                            if k < Ka:
                                r = bi * Ka + k
                                ck = gmA[:, r * P2W:r * P2W + 65]
                            else:
                                r = bi * Kb + (k - Ka)
                                ck = gmB[:, r * P2W:r * P2W + 65]
                            q = bi * K + k
                            nc.tensor.matmul(
                                ps[:], lhsT=S_sb[:, q * 128:(q + 1) * 128],
                                rhs=ck, start=(k == 0), stop=(k == K - 1))
                        nc.vector.tensor_copy(
                            out=oball[:, bi * 65:(bi + 1) * 65], in_=ps[:])
                    rd = sml2.tile([128, G], F32, tag="rd2")
                    rd3 = rd[:].rearrange("p (g o) -> p g o", o=1)
                    nc.vector.tensor_scalar_add(rd3, ob4[:, :, 64:65], EPS)
                    nc.vector.reciprocal(rd[:], rd[:])
                    ob = obp.tile([128, G * 64], F32, tag="ob")
                    ob3 = ob[:].rearrange("p (g w) -> p g w", g=G)
                    nc.vector.tensor_tensor(
                        out=ob3, in0=ob4[:, :, 0:64],
                        in1=rd3.to_broadcast([128, G, 64]), op=mm.mult)
                    nc.vector.tensor_tensor(
                        out=ob3, in0=ob3,
                        in1=B2t[:].rearrange("p (o w) -> p o w",
                                             o=1).to_broadcast([128, G, 64]),
                        op=mm.add)
                    n0 = sb * G * BLK
                    nc.sync.dma_start(
                        out=OUT[n0:n0 + G * BLK, :].rearrange(
                            "(g n) w -> n g w", n=BLK),
                        in_=ob[:BLK, :].rearrange("n (g w) -> n g w", g=G))

    nc.compile()
    return nc


def _run(inputs, sim=False, trace=False):
    in_maps, prm = _host_prep(**inputs)
    nc = _build_program(prm)
    n_cores = prm["n_cores"]
    if sim:
        from concourse.bass_interp import MultiCoreSim
        ms = MultiCoreSim(nc, num_cores=n_cores)
        for c in range(n_cores):
            for k, v in in_maps[c].items():
                ms.cores[c].tensor(k)[:] = v
        ms.simulate()
        outs = [np.array(ms.cores[c].tensor("out")) for c in range(n_cores)]
        got = np.concatenate(outs, axis=0)
        res = None
    else:
        from concourse.bass_utils import run_bass_kernel_spmd
        res = run_bass_kernel_spmd(nc, in_maps, core_ids=list(range(n_cores)),
                                   trace=trace)
        outs = [res.results[c]["out"] for c in range(n_cores)]
        got = np.concatenate(outs, axis=0)
    full = np.empty_like(got)
    full[prm["perm"]] = got
    return full, res


def kernel(**inputs):
    out, _ = _run({k: np.asarray(v) for k, v in inputs.items()})
    return out
